# revision 1
# baseline (speedup 1.0000x reference)
# Bass/Tile TRN2 kernel for nn_BiasedCrossDecoderLayer (dense cross-attention
# transformer decoder layer), SPMD over 8 NeuronCores.
#
# Sharding: core c -> batch b = c//4, head-group hg = c%4 (4 of 16 heads =
# 256 of 1024 qkv feature dims).  Attention is head-parallel; the
# out-projection produces partial sums which are ReduceScattered (along the
# query axis) within each 4-core batch group; the FFN then runs
# sequence-parallel on each core's 256-query slice with the full 4096 hidden.
#
# LayerNorms are folded into the weights host-side:
#   q = LN(x;g,b) @ Wq.T + pq  ==  LN0(x) @ Wq'.T + bias'
#   with Wq' = wq*g (and the 1/sqrt(hd) attention scale folded into Wq'/bias'),
#   bias' = wq@b + pq, and LN0 the pure normalize (x - m) / std.
# In feature-major layout (activations stored transposed, [feature, token]):
#   qT = rB * (Wq' @ xT_raw + ADJ)
#   ADJ[o,t] = -rowsum(Wq')[o]*m[t] + bias'[o]*std[t]    (rank-2, emitted as a
#              K=2 matmul appended to the same PSUM accumulation group)
# so each projection needs exactly one DVE pass.  V is produced token-major
# (the PV matmul needs it) with the analogous fold.  No on-chip transposes.
#
# Attention runs in the transposed [s, q] layout (mask is pre-transposed on
# the host).  The softmax denominator comes from a ones-column appended to
# the V stationary operand (M=65 matmul); 1/sum is applied after the PV
# accumulation via a K=1 broadcast matmul + one DVE multiply.
#
# All matmuls read operands as float32r (fp32 bits at ~fp22 precision, full
# PE rate; plain fp32 would be 4x slower).

import os
import sys

import numpy as np

sys.path.insert(0, "/opt/trn_rl_repo")

import concourse.bass as bass  # noqa: E402
import concourse.mybir as mybir  # noqa: E402
import concourse.tile as tile  # noqa: E402
from concourse import bacc  # noqa: E402

F32 = mybir.dt.float32
F32R = mybir.dt.float32r
AF = mybir.ActivationFunctionType
ALU = mybir.AluOpType

B, Q, S, D, H = 2, 1024, 2048, 1024, 16
HD = D // H       # 64
FF = 4 * D
EPS = 1e-5
NCORES = 8
NH = 4            # heads per core
FC = NH * HD      # 256 qkv feature dims per core
QS = Q // 4       # 256-query slice per core after reduce-scatter
P = 128
KX = D // P       # 8 k-tiles over the model dim

REPLICA_GROUPS = [[0, 1, 2, 3], [4, 5, 6, 7]]

LAST_RESULT = None  # BassKernelResults of the most recent run (for test.py)


def _r(ap):
    """View an fp32 AP as float32r for full-rate PE matmuls."""
    return ap.bitcast(F32R)


def build_nc():
    nc = bacc.Bacc(
        "TRN2",
        target_bir_lowering=False,
        debug=False,
        num_devices=NCORES,
        name="biased_cross_decoder",
    )

    # ---- DRAM I/O (per-core shards; same program on all cores) ----
    d = {}
    d["ones_t"] = nc.dram_tensor("ones_t", [P, P], F32R, kind="ExternalInput").ap()
    d["xT"] = nc.dram_tensor("xT", [D, Q], F32R, kind="ExternalInput").ap()
    d["zT"] = nc.dram_tensor("zT", [D, S], F32R, kind="ExternalInput").ap()
    d["maskT"] = nc.dram_tensor("maskT", [NH, S, Q], F32, kind="ExternalInput").ap()
    d["wqT"] = nc.dram_tensor("wqT", [P, KX, FC], F32R, kind="ExternalInput").ap()
    d["wkT"] = nc.dram_tensor("wkT", [P, KX, FC], F32R, kind="ExternalInput").ap()
    d["wvT"] = nc.dram_tensor("wvT", [P, KX, FC], F32R, kind="ExternalInput").ap()
    d["adjq"] = nc.dram_tensor("adjq", [2, FC], F32R, kind="ExternalInput").ap()
    d["adjk"] = nc.dram_tensor("adjk", [2, FC], F32R, kind="ExternalInput").ap()
    d["adjv"] = nc.dram_tensor("adjv", [2, FC], F32R, kind="ExternalInput").ap()
    d["owh"] = nc.dram_tensor("owh", [HD, NH, D], F32R, kind="ExternalInput").ap()
    d["outb"] = nc.dram_tensor("outb", [D], F32, kind="ExternalInput").ap()
    d["xq"] = nc.dram_tensor("xq", [D, QS], F32, kind="ExternalInput").ap()
    d["w1p"] = nc.dram_tensor("w1p", [FF // P, P, KX, P], F32R, kind="ExternalInput").ap()
    d["adjf"] = nc.dram_tensor("adjf", [2, FF], F32R, kind="ExternalInput").ap()
    d["w2T"] = nc.dram_tensor("w2T", [FF, D], F32R, kind="ExternalInput").ap()
    d["b2"] = nc.dram_tensor("b2", [D], F32, kind="ExternalInput").ap()
    d["out"] = nc.dram_tensor("out", [D, QS], F32, kind="ExternalOutput").ap()

    with tile.TileContext(nc) as tc:
        build_tile_program(tc, nc, d)
    nc.compile()   # bacc passes: wait splitting, ldweights fusion, reg alloc
    return nc


class _Pool:
    """Keeps the tile_pool context manager alive; allows explicit close."""

    def __init__(self, cm):
        self._cm = cm
        self.pool = cm.__enter__()

    def tile(self, *a, **kw):
        kw.setdefault("name", kw.get("tag") or "t")
        return self.pool.tile(*a, **kw)

    def close(self):
        self._cm.__exit__(None, None, None)


def build_tile_program(tc, nc, d):
    # ---------------- persistent constants ----------------
    const = _Pool(tc.tile_pool(name="const", bufs=1))
    dram = _Pool(tc.tile_pool(name="dram", bufs=1, space="DRAM"))

    # ones come from DRAM: DVE memset cannot write fp32r (ISA limitation)
    ones_sb = const.tile([P, P], F32R, tag="ones_sb")
    nc.sync.dma_start(ones_sb, d["ones_t"])
    ones_col = ones_sb[:, 0:1]                  # lhsT for column sums
    ones_row = ones_sb[0:1, :]                  # lhsT for partition broadcasts
    ones65 = ones_sb                            # row 64 used as base-64 lhsT
    eps_t = const.tile([1, 1], F32, tag="eps")
    nc.vector.memset(eps_t, EPS)
    outb_col = const.tile([P, KX], F32, tag="outb_col")
    nc.sync.dma_start(outb_col, d["outb"].rearrange("(o p) -> p o", p=P))
    b2_col = const.tile([P, KX], F32, tag="b2_col")
    nc.sync.dma_start(b2_col, d["b2"].rearrange("(o p) -> p o", p=P))
    rz_col = const.tile([P, S // P], F32R, tag="rz_col")    # rstd_z token-striped

    rs_scr = dram.tile([1, S], F32R, tag="rs_scr")          # row restripe bounce

    # ---------------- long-lived right-side pools ----------------
    # (separate allocator stack: released in LIFO order attp after qkv)
    pool_att = _Pool(tc.tile_pool(name="attp", bufs=1, side="right"))
    pool_qkv = _Pool(tc.tile_pool(name="qkv", bufs=1, side="right"))

    # ---------------- phase A/B scratch pools (left stack) ----------------
    pool_rows = _Pool(tc.tile_pool(name="rows", bufs=2))
    pool_sq = _Pool(tc.tile_pool(name="sq", bufs=3))
    pool_adj = _Pool(tc.tile_pool(name="adj", bufs=1))
    pool_z = _Pool(tc.tile_pool(name="pz", bufs=1))
    pool_x = _Pool(tc.tile_pool(name="px", bufs=1))

    # stat row tiles ([2, T] lhsT/rhs operands for the rank-2 ADJ matmuls)
    adjx = pool_adj.tile([2, Q], F32R, tag="adjx")          # [mx ; stdx]
    adjz = pool_adj.tile([2, S], F32R, tag="adjz")          # [mz ; stdz]

    xT = pool_x.tile([P, KX, Q], F32R, tag="xT")
    for ch in range(2):
        for k in range(KX):
            nc.sync.dma_start(xT[:, k, ch * 512:(ch + 1) * 512],
                              d["xT"][k * P:(k + 1) * P, ch * 512:(ch + 1) * 512])
    zT = pool_z.tile([P, KX, S], F32R, tag="zT")
    for ch in range(4):
        for k in range(KX):
            nc.sync.dma_start(zT[:, k, ch * 512:(ch + 1) * 512],
                              d["zT"][k * P:(k + 1) * P, ch * 512:(ch + 1) * 512])

    def ln_stats2(aT, T, adj, rB, ps_stats, ps_bcp, scr=None):
        """Per 512-token chunk: LN stats -> adj=[mean;std] rows, broadcast
        1/std into rB [P,T]; optionally stash 1/std to scr (DRAM)."""
        for ch in range(T // 512):
            sl = slice(ch * 512, (ch + 1) * 512)
            ps_sum = ps_stats.tile([1, 512], F32, name="ps_sum", tag="ps_sum")
            ps_ssq = ps_stats.tile([1, 512], F32, name="ps_ssq", tag="ps_ssq")
            for k in range(KX):
                nc.tensor.matmul(ps_sum, _r(ones_col), _r(aT[:, k, sl]),
                                 start=(k == 0), stop=(k == KX - 1))
                sq = pool_sq.tile([P, 512], F32R, name="sq", tag="sq")
                nc.scalar.square(sq, aT[:, k, sl])
                nc.tensor.matmul(ps_ssq, _r(ones_col), _r(sq),
                                 start=(k == 0), stop=(k == KX - 1))
            e2 = pool_rows.tile([1, 512], F32, name="e2", tag="e2")
            m2 = pool_rows.tile([1, 512], F32, name="m2", tag="m2")
            inv = pool_rows.tile([1, 512], F32R, name="inv", tag="inv")
            rr = pool_rows.tile([1, 512], F32R, name="rr", tag="rr")
            nc.vector.tensor_scalar_mul(adj[0:1, sl], ps_sum, 1.0 / D)  # mean
            nc.vector.tensor_scalar_mul(e2, ps_ssq, 1.0 / D)            # E[x^2]
            nc.vector.tensor_mul(m2, adj[0:1, sl], adj[0:1, sl])
            nc.vector.tensor_sub(e2, e2, m2)                            # var
            nc.scalar.activation(inv, e2, AF.Sqrt, bias=eps_t[0:1])     # std
            with nc.allow_low_precision(reason="fp32r rounding of 1/std"):
                nc.vector.reciprocal(rr, inv)
            nc.sync.dma_start(adj[1:2, sl], inv)   # cross-partition row move
            bc = ps_bcp.tile([P, 512], F32, name="bc", tag="bc")
            nc.tensor.matmul(bc, _r(ones_row), _r(rr))
            nc.scalar.copy(rB[:, sl], bc)
            if scr is not None:
                nc.sync.dma_start(scr[0:1, sl], rr)

    # ---- x statistics + broadcast of rx ----
    pool_bcx = _Pool(tc.tile_pool(name="bcx", bufs=1))
    rxB = pool_bcx.tile([P, Q], F32, tag="rxB")
    with tc.tile_pool(name="ps_sx", bufs=2, space="PSUM") as ps_sx, \
         tc.tile_pool(name="ps_bcx", bufs=2, space="PSUM") as ps_bcx:
        ln_stats2(xT, Q, adjx, rxB, ps_sx, ps_bcx)

    # ---- q projection (feature-major) ----
    qT = pool_qkv.tile([P, FC // P, Q], F32R, tag="qT")   # includes 1/8 scale
    kT = pool_qkv.tile([P, FC // P, S], F32R, tag="kT")
    v_sb = pool_qkv.tile([P, S // P, NH, HD + 1], F32R, tag="v_sb")

    with tc.tile_pool(name="wq", bufs=1) as pool_wq, \
         tc.tile_pool(name="ps_q", bufs=3, space="PSUM") as ps_qk:
        wq_sb = pool_wq.tile([P, KX, FC], F32R, tag="wq_sb")
        nc.sync.dma_start(wq_sb, d["wqT"])
        adjq_w = pool_wq.tile([2, FC], F32R, tag="adjq_w")
        nc.sync.dma_start(adjq_w, d["adjq"])
        for m in range(FC // P):
            for ch in range(Q // 512):
                sl = slice(ch * 512, (ch + 1) * 512)
                ps = ps_qk.tile([P, 512], F32, name="ps_qk_t", tag="ps_qk_t")
                for k in range(KX):
                    nc.tensor.matmul(ps, _r(wq_sb[:, k, m * P:(m + 1) * P]),
                                     _r(xT[:, k, sl]), start=(k == 0), stop=False)
                nc.tensor.matmul(ps, _r(adjq_w[:, m * P:(m + 1) * P]),
                                 _r(adjx[:, sl]), start=False, stop=True)
                nc.vector.tensor_mul(qT[:, m, sl], ps, rxB[:, sl])

    pool_bcx.close()
    pool_x.close()

    # ---- z statistics + broadcast of rz + restripe rz to columns ----
    pool_bcz = _Pool(tc.tile_pool(name="bcz", bufs=1))
    rzB = pool_bcz.tile([P, S], F32, tag="rzB")
    with tc.tile_pool(name="ps_sz", bufs=2, space="PSUM") as ps_sz, \
         tc.tile_pool(name="ps_bcz", bufs=2, space="PSUM") as ps_bcz:
        ln_stats2(zT, S, adjz, rzB, ps_sz, ps_bcz, scr=rs_scr)
    nc.sync.dma_start(rz_col, rs_scr.rearrange("a (i p) -> (a p) i", p=P))

    # ---- k / v projections ----
    # softmax-denominator ones column (DMA: DVE memset cannot write fp32r)
    nc.sync.dma_start(
        v_sb[:, :, :, HD:HD + 1],
        d["ones_t"][:, 0:S // P * NH].rearrange("p (a b c) -> p a b c",
                                                a=S // P, c=1))

    with tc.tile_pool(name="wkv", bufs=1) as pool_wkv, \
         tc.tile_pool(name="ps_k", bufs=3, space="PSUM") as ps_qk, \
         tc.tile_pool(name="ps_v", bufs=2, space="PSUM") as ps_v:
        wk_sb = pool_wkv.tile([P, KX, FC], F32R, tag="wk_sb")
        nc.sync.dma_start(wk_sb, d["wkT"])
        wv_sb = pool_wkv.tile([P, KX, FC], F32R, tag="wv_sb")
        nc.sync.dma_start(wv_sb, d["wvT"])
        adjk_w = pool_wkv.tile([2, FC], F32R, tag="adjk_w")
        nc.sync.dma_start(adjk_w, d["adjk"])
        adjv_w = pool_wkv.tile([2, FC], F32R, tag="adjv_w")
        nc.sync.dma_start(adjv_w, d["adjv"])

        for m in range(FC // P):
            for ch in range(S // 512):
                sl = slice(ch * 512, (ch + 1) * 512)
                ps = ps_qk.tile([P, 512], F32, name="ps_qk_t", tag="ps_qk_t")
                for k in range(KX):
                    nc.tensor.matmul(ps, _r(wk_sb[:, k, m * P:(m + 1) * P]),
                                     _r(zT[:, k, sl]), start=(k == 0), stop=False)
                nc.tensor.matmul(ps, _r(adjk_w[:, m * P:(m + 1) * P]),
                                 _r(adjz[:, sl]), start=False, stop=True)
                nc.vector.tensor_mul(kT[:, m, sl], ps, rzB[:, sl])

        # v: token-major; ADJ lhsT = [mz ; stdz], rhs = [-rowsum(Wv') ; biasv']
        for t in range(S // P):
            ps = ps_v.tile([P, FC], F32, name="ps_v_t", tag="ps_v_t")
            for k in range(KX):
                nc.tensor.matmul(ps, _r(zT[:, k, t * P:(t + 1) * P]),
                                 _r(wv_sb[:, k, :]), start=(k == 0), stop=False)
            nc.tensor.matmul(ps, _r(adjz[:, t * P:(t + 1) * P]), _r(adjv_w),
                             start=False, stop=True)
            nc.vector.tensor_scalar_mul(
                v_sb[:, t, :, 0:HD],
                ps.rearrange("p (h e) -> p h e", h=NH),
                rz_col[:, t:t + 1].bitcast(F32))

    pool_bcz.close()
    pool_z.close()
    pool_adj.close()
    pool_sq.close()
    pool_rows.close()

    # =================== attention ===================
    att64 = [pool_att.tile([HD, Q], F32R, name=f"att64_{h}", tag=f"att64_{h}")
             for h in range(NH)]

    with tc.tile_pool(name="mask", bufs=16) as pool_mask, \
         tc.tile_pool(name="probs", bufs=5) as pool_probs, \
         tc.tile_pool(name="nbc", bufs=2) as pool_nbc, \
         tc.tile_pool(name="rrow", bufs=2) as pool_rrow, \
         tc.tile_pool(name="ps_lg", bufs=2, space="PSUM") as ps_lg, \
         tc.tile_pool(name="ps_att", bufs=1, space="PSUM") as ps_att, \
         tc.tile_pool(name="ps_nbc", bufs=1, space="PSUM") as ps_nbc:

        for h in range(NH):
            ht, ho = h // 2, HD * (h % 2)
            att_ps = ps_att.tile([HD + 1, Q], F32, name="att_ps", tag="att_ps")
            for st in range(S // P):
                mk = pool_mask.tile([P, Q], F32, name="mk", tag="mk")
                nc.sync.dma_start(mk, d["maskT"][h, st * P:(st + 1) * P, :])
                pr = pool_probs.tile([P, Q], F32R, name="pr", tag="pr")
                lg = ps_lg.tile([P, Q], F32, name="lg", tag="lg")
                for ch in range(Q // 512):
                    sl = slice(ch * 512, (ch + 1) * 512)
                    nc.tensor.matmul(
                        lg[:, sl],
                        _r(kT[ho:ho + HD, ht, st * P:(st + 1) * P]),
                        _r(qT[ho:ho + HD, ht, sl]))
                nc.vector.tensor_add(pr, lg, mk)
                nc.scalar.activation(pr, pr, AF.Exp)
                for ch in range(Q // 512):
                    sl = slice(ch * 512, (ch + 1) * 512)
                    nc.tensor.matmul(att_ps[:, sl], _r(v_sb[:, st, h, :]),
                                     _r(pr[:, sl]),
                                     start=(st == 0), stop=(st == S // P - 1))
            # normalize: att[0:64] * broadcast(1 / att[64])
            rr = pool_rrow.tile([HD + 1, Q], F32R, name="rr", tag="rr")
            with nc.allow_low_precision(reason="fp32r rounding of 1/sum"):
                nc.vector.reciprocal(rr[HD:HD + 1, :], att_ps[HD:HD + 1, :])
            nbc = pool_nbc.tile([HD, Q], F32, name="nbc_t", tag="nbc_t")
            for ch in range(Q // 512):
                sl = slice(ch * 512, (ch + 1) * 512)
                bc = ps_nbc.tile([HD, 512], F32, name="bc2", tag="bc2")
                nc.tensor.matmul(bc, _r(ones65[HD:HD + 1, 0:HD]),
                                 _r(rr[HD:HD + 1, sl]))
                nc.scalar.copy(nbc[:, sl], bc)
            nc.vector.tensor_mul(att64[h], att_ps[0:HD, :], nbc)

    pool_qkv.close()

    # =================== out-projection + ReduceScatter ===================
    DH = D // 2
    rs_in = [dram.tile([4, DH, QS], F32, name=f"rs_in{i}", tag=f"rs_in{i}")
             for i in range(2)]
    rs_out = [dram.tile([DH, QS], F32, name=f"rs_out{i}", tag=f"rs_out{i}")
              for i in range(2)]

    with tc.tile_pool(name="ow", bufs=1) as pool_ow, \
         tc.tile_pool(name="osb", bufs=3) as pool_osb, \
         tc.tile_pool(name="ps_o", bufs=3, space="PSUM") as ps_o:
        ow_sb = pool_ow.tile([HD, NH, D], F32R, tag="ow_sb")
        nc.sync.dma_start(ow_sb, d["owh"])
        for half in range(2):
            for mi in range(D // P // 2):
                m = half * (D // P // 2) + mi
                for ch in range(Q // 512):
                    sl = slice(ch * 512, (ch + 1) * 512)
                    ps = ps_o.tile([P, 512], F32, name="ps_o_t", tag="ps_o_t")
                    for h in range(NH):
                        nc.tensor.matmul(ps, _r(ow_sb[:, h, m * P:(m + 1) * P]),
                                         _r(att64[h][:, sl]),
                                         start=(h == 0), stop=(h == NH - 1))
                    ot = pool_osb.tile([P, 512], F32, name="ot", tag="ot")
                    nc.scalar.copy(ot, ps)
                    for r2 in range(2):
                        nc.sync.dma_start(
                            rs_in[half][2 * ch + r2, mi * P:(mi + 1) * P, :],
                            ot[:, r2 * QS:(r2 + 1) * QS])
            # launch this half's reduce-scatter while the other half computes
            nc.gpsimd.collective_compute(
                "ReduceScatter",
                ALU.add,
                replica_groups=REPLICA_GROUPS,
                ins=[rs_in[half].opt()],
                outs=[rs_out[half].opt()],
            )

    pool_att.close()

    # =================== residual + FFN (sequence-parallel) ===================
    with tc.tile_pool(name="ffn", bufs=1) as pool_f, \
         tc.tile_pool(name="w1s", bufs=8) as pool_w1, \
         tc.tile_pool(name="w2s", bufs=8) as pool_w2, \
         tc.tile_pool(name="gact", bufs=3) as pool_g, \
         tc.tile_pool(name="rsld", bufs=3) as pool_rsld, \
         tc.tile_pool(name="yout", bufs=3) as pool_yo, \
         tc.tile_pool(name="ps_f", bufs=2, space="PSUM") as ps_f, \
         tc.tile_pool(name="ps_y2", bufs=1, space="PSUM") as ps_y2:

        y1T = pool_f.tile([P, KX, QS], F32R, tag="y1T")
        adjy = pool_f.tile([2, QS], F32R, tag="adjy")      # [my ; stdy]
        ry_row = pool_f.tile([1, QS], F32R, tag="ry_row")
        ryB = pool_f.tile([P, QS], F32, tag="ryB")
        adjf_w = pool_f.tile([2, FF], F32R, tag="adjf_w")
        nc.sync.dma_start(adjf_w, d["adjf"])

        # y1 = RS(out-proj partials) + x_slice + out_b   (feature-major)
        # gpsimd DMAs: keep the HWDGE queues free for weight prefetch while
        # the collective is still in flight
        for m in range(KX):
            half, mi = m // (KX // 2), m % (KX // 2)
            rst = pool_rsld.tile([P, QS], F32, name="rst", tag="rst")
            nc.gpsimd.dma_start(rst, rs_out[half][mi * P:(mi + 1) * P, :])
            xqt = pool_rsld.tile([P, QS], F32, name="xqt", tag="xqt")
            nc.gpsimd.dma_start(xqt, d["xq"][m * P:(m + 1) * P, :])
            nc.vector.scalar_tensor_tensor(
                out=y1T[:, m, :], in0=rst, scalar=outb_col[:, m:m + 1],
                in1=xqt, op0=ALU.add, op1=ALU.add)

        # y1 LN stats
        with tc.tile_pool(name="ps_yst", bufs=1, space="PSUM") as ps_yst:
            e2_row = pool_f.tile([1, QS], F32, tag="e2y_row")
            m2_row = pool_f.tile([1, QS], F32, tag="m2y_row")
            inv_row = pool_f.tile([1, QS], F32R, tag="invy_row")
            ps_sum = ps_yst.tile([1, QS], F32, name="ps_sum2", tag="ps_sum2")
            ps_ssq = ps_yst.tile([1, QS], F32, name="ps_ssq2", tag="ps_ssq2")
            for k in range(KX):
                nc.tensor.matmul(ps_sum, _r(ones_col), _r(y1T[:, k, :]),
                                 start=(k == 0), stop=(k == KX - 1))
                sq = pool_g.tile([P, QS], F32R, name="ysq", tag="gt")
                nc.scalar.square(sq, y1T[:, k, :])
                nc.tensor.matmul(ps_ssq, _r(ones_col), _r(sq),
                                 start=(k == 0), stop=(k == KX - 1))
            nc.vector.tensor_scalar_mul(adjy[0:1, :], ps_sum, 1.0 / D)
            nc.vector.tensor_scalar_mul(e2_row, ps_ssq, 1.0 / D)
            nc.vector.tensor_mul(m2_row, adjy[0:1, :], adjy[0:1, :])
            nc.vector.tensor_sub(e2_row, e2_row, m2_row)
            nc.scalar.activation(inv_row, e2_row, AF.Sqrt, bias=eps_t[0:1])
            with nc.allow_low_precision(reason="fp32r rounding of 1/std"):
                nc.vector.reciprocal(ry_row, inv_row)
            nc.sync.dma_start(adjy[1:2, :], inv_row)
            bc = ps_f.tile([P, QS], F32, name="bc3", tag="ps_f_t")
            nc.tensor.matmul(bc, _r(ones_row), _r(ry_row))
            nc.scalar.copy(ryB, bc)

        # ff1 + exact gelu into one persistent [P, 32, QS] activation tile
        g_sb = pool_f.tile([P, FF // P, QS], F32R, tag="g_sb")
        for j in range(FF // P):
            w1b = pool_w1.tile([P, KX, P], F32R, name="w1b", tag="w1b")
            nc.sync.dma_start(w1b, d["w1p"][j])
            ps = ps_f.tile([P, QS], F32, name="ps_f_t", tag="ps_f_t")
            for k in range(KX):
                nc.tensor.matmul(ps, _r(w1b[:, k, :]), _r(y1T[:, k, :]),
                                 start=(k == 0), stop=False)
            nc.tensor.matmul(ps, _r(adjf_w[:, j * P:(j + 1) * P]), _r(adjy),
                             start=False, stop=True)
            nc.vector.tensor_mul(g_sb[:, j, :], ps, ryB)   # ff1 = ry*(raw+adj)
            nc.scalar.activation(g_sb[:, j, :], g_sb[:, j, :], AF.Gelu)

        # ff2 in two half-D passes; each output m-tile gets a full psum bank
        for half in range(2):
            y2a = [ps_y2.tile([P, QS], F32, name=f"y2a_{i}", tag=f"y2a_{i}",
                              bufs=1) for i in range(4)]
            for j in range(FF // P):
                w2b = pool_w2.tile([P, D // 2], F32R, name="w2b", tag="w2b")
                nc.sync.dma_start(
                    w2b, d["w2T"][j * P:(j + 1) * P,
                                  half * (D // 2):(half + 1) * (D // 2)])
                for mi in range(4):
                    nc.tensor.matmul(y2a[mi], _r(w2b[:, mi * P:(mi + 1) * P]),
                                     _r(g_sb[:, j, :]),
                                     start=(j == 0), stop=(j == FF // P - 1))
            for mi in range(4):
                m = half * 4 + mi
                yt = pool_yo.tile([P, QS], F32, name="yt", tag="yt")
                nc.vector.scalar_tensor_tensor(
                    out=yt, in0=y2a[mi], scalar=b2_col[:, m:m + 1],
                    in1=y1T[:, m, :], op0=ALU.add, op1=ALU.add)
                nc.sync.dma_start(d["out"][m * P:(m + 1) * P, :], yt)

    const.close()
    dram.close()


def host_prep(inputs):
    """Fold layernorm gains/biases into weights; build the 8 per-core shards."""
    f32 = np.float32
    x = np.asarray(inputs["x"], f32)
    z = np.asarray(inputs["z"], f32)
    mask = np.asarray(inputs["attn_mask"], f32)
    gq = np.asarray(inputs["gq"], np.float64)
    bq = np.asarray(inputs["bq"], np.float64)
    gkv = np.asarray(inputs["gkv"], np.float64)
    bkv = np.asarray(inputs["bkv"], np.float64)
    gff = np.asarray(inputs["gff"], np.float64)
    bff = np.asarray(inputs["bff"], np.float64)
    ipw = np.asarray(inputs["in_proj_w"], np.float64)
    ipb = np.asarray(inputs["in_proj_b"], np.float64)
    out_w = np.asarray(inputs["out_w"], f32)
    out_b = np.asarray(inputs["out_b"], f32)
    w1 = np.asarray(inputs["w1"], np.float64)
    b1 = np.asarray(inputs["b1"], np.float64)
    w2 = np.asarray(inputs["w2"], f32)
    b2 = np.asarray(inputs["b2"], f32)

    wq, wk, wv = ipw[:D], ipw[D:2 * D], ipw[2 * D:]
    pq, pk, pv = ipb[:D], ipb[D:2 * D], ipb[2 * D:]
    scale = 1.0 / np.sqrt(HD)
    wq2 = (wq * gq[None, :]) * scale
    pq2 = (wq @ bq + pq) * scale
    wk2 = wk * gkv[None, :]
    pk2 = wk @ bkv + pk
    wv2 = wv * gkv[None, :]
    pv2 = wv @ bkv + pv
    w12 = w1 * gff[None, :]
    b12 = w1 @ bff + b1

    w1T = np.ascontiguousarray(w12.T.astype(f32))                    # (D, FF)
    # packed so each hidden-block's [P, KX, P] lhsT tile set is contiguous
    w1p = np.ascontiguousarray(
        w1T.reshape(KX, P, FF // P, P).transpose(2, 1, 0, 3))
    adjf = np.ascontiguousarray(
        np.stack([-w12.sum(1), b12]).astype(f32))                    # (2, FF)
    w2T = np.ascontiguousarray(w2.T)                                 # (FF, D)

    def pack_kxf(wT):  # (D, FC) -> (P, D//P, FC)
        return np.ascontiguousarray(wT.reshape(KX, P, FC).transpose(1, 0, 2))

    in_maps = []
    for c in range(NCORES):
        b, hg = c // 4, c % 4
        fs = slice(FC * hg, FC * hg + FC)
        qs = slice(QS * (c % 4), QS * (c % 4) + QS)
        xTb = np.ascontiguousarray(x[b].T)                           # (D, Q)
        in_maps.append({
            "ones_t": np.ones((P, P), f32),
            "xT": xTb,
            "zT": np.ascontiguousarray(z[b].T),
            "maskT": np.ascontiguousarray(
                mask[16 * b + NH * hg:16 * b + NH * hg + NH].transpose(0, 2, 1)),
            "wqT": pack_kxf(np.ascontiguousarray(wq2[fs].T.astype(f32))),
            "wkT": pack_kxf(np.ascontiguousarray(wk2[fs].T.astype(f32))),
            "wvT": pack_kxf(np.ascontiguousarray(wv2[fs].T.astype(f32))),
            "adjq": np.ascontiguousarray(
                np.stack([-wq2[fs].sum(1), pq2[fs]]).astype(f32)),
            "adjk": np.ascontiguousarray(
                np.stack([-wk2[fs].sum(1), pk2[fs]]).astype(f32)),
            "adjv": np.ascontiguousarray(
                np.stack([-wv2[fs].sum(1), pv2[fs]]).astype(f32)),
            "owh": np.ascontiguousarray(
                out_w[:, fs].T.reshape(NH, HD, D).transpose(1, 0, 2)),
            "outb": out_b,
            "xq": np.ascontiguousarray(xTb[:, qs]),
            "w1p": w1p,
            "adjf": adjf,
            "w2T": w2T,
            "b2": b2,
        })
    return in_maps


_NC_CACHE = None


def kernel(**inputs) -> np.ndarray:
    global _NC_CACHE, LAST_RESULT
    from concourse.bass_utils import run_bass_kernel_spmd

    in_maps = host_prep(inputs)
    if _NC_CACHE is None:
        _NC_CACHE = build_nc()
    res = run_bass_kernel_spmd(
        _NC_CACHE, in_maps, core_ids=list(range(NCORES)),
        trace=bool(os.environ.get("BASS_TRACE")),
    )
    LAST_RESULT = res
    out = np.empty((B, Q, D), np.float32)
    for c in range(NCORES):
        b = c // 4
        qs = slice(QS * (c % 4), QS * (c % 4) + QS)
        out[b, qs, :] = res.results[c]["out"].T
    return out



# revision 41
# speedup vs baseline: 1.1833x; 1.1833x over previous
# Bass/Tile TRN2 kernel for nn_BiasedCrossDecoderLayer (dense cross-attention
# transformer decoder layer), SPMD over 8 NeuronCores.
#
# Sharding: core c -> batch b = c//4, head-group hg = c%4 (4 of 16 heads =
# 256 of 1024 qkv feature dims).  Attention is head-parallel; the
# out-projection produces partial sums which are ReduceScattered (along the
# query axis, in 4 pipelined bf16 chunks) within each 4-core batch group;
# the FFN then runs sequence-parallel on each core's 256-query slice.
#
# All matmul operands are bf16 (fp32 PSUM accumulation): on TRN2 hardware
# fp32r streams at ~1.6-2 cycles/row while bf16 streams at 1.0, and bf16
# halves mask/weight HBM traffic.  LayerNorms are folded into the weights
# host-side (see ADJ rank-2 matmul trick below); the adj matmuls stay fp32r.
#
#   q = LN(x;g,b) @ Wq.T + pq  ==  LN0(x) @ Wq'.T + bias'
#   qT = rB * (Wq' @ xT_raw + ADJ)
#   ADJ[o,t] = -rowsum(Wq')[o]*m[t] + bias'[o]*std[t]    (rank-2 matmul
#              appended to the same PSUM accumulation group)
#
# Attention runs in the transposed [s, q] layout (mask pre-transposed on the
# host).  The softmax denominator comes from a ones-column appended to the V
# stationary operand (M=65 matmul); 1/sum via reciprocal_approx_fast.  The
# mask+logits add is split between the DVE and PE (identity-matmul accumulate
# into the logits PSUM group) to balance engines; the (logits -> add -> exp ->
# PV) chain is software-pipelined by 2 steps on the in-order PE queue.

import os
import sys

import numpy as np

sys.path.insert(0, "/opt/trn_rl_repo")

import ml_dtypes  # noqa: E402

import concourse.bass as bass  # noqa: E402
import concourse.mybir as mybir  # noqa: E402
import concourse.tile as tile  # noqa: E402
from concourse import bacc  # noqa: E402

F32 = mybir.dt.float32
F32R = mybir.dt.float32r
BF16 = mybir.dt.bfloat16
AF = mybir.ActivationFunctionType
ALU = mybir.AluOpType
BF = ml_dtypes.bfloat16

B, Q, S, D, H = 2, 1024, 2048, 1024, 16
HD = D // H       # 64
FF = 4 * D
EPS = 1e-5
NCORES = 8
NH = 4            # heads per core
FC = NH * HD      # 256 qkv feature dims per core
QS = Q // 4       # 256-query slice per core after reduce-scatter
P = 128
KX = D // P       # 8 k-tiles over the model dim
NST = S // P      # 16 s-tiles
RSCH = 2          # reduce-scatter chunks (collective overhead is ~10us each)

REPLICA_GROUPS = [[0, 1, 2, 3], [4, 5, 6, 7]]

LAST_RESULT = None  # BassKernelResults of the most recent run (for test.py)


def _r(ap):
    """View an fp32 AP as float32r for full-rate PE matmuls."""
    return ap.bitcast(F32R)


def build_nc():
    nc = bacc.Bacc(
        "TRN2",
        target_bir_lowering=False,
        debug=False,
        num_devices=NCORES,
        name="biased_cross_decoder",
    )

    # ---- DRAM I/O (per-core shards; same program on all cores) ----
    d = {}
    d["ones_bf"] = nc.dram_tensor("ones_bf", [P, P], BF16, kind="ExternalInput").ap()
    d["ident_bf"] = nc.dram_tensor("ident_bf", [P, P], BF16, kind="ExternalInput").ap()
    d["xT"] = nc.dram_tensor("xT", [D, Q], BF16, kind="ExternalInput").ap()
    d["zT"] = nc.dram_tensor("zT", [D, S], BF16, kind="ExternalInput").ap()
    d["maskT"] = nc.dram_tensor("maskT", [NH, S, Q], BF16, kind="ExternalInput").ap()
    d["wqT"] = nc.dram_tensor("wqT", [P, KX, FC], BF16, kind="ExternalInput").ap()
    d["wkT"] = nc.dram_tensor("wkT", [P, KX, FC], BF16, kind="ExternalInput").ap()
    d["wvT"] = nc.dram_tensor("wvT", [P, KX, FC], BF16, kind="ExternalInput").ap()
    d["adjq"] = nc.dram_tensor("adjq", [2, FC], F32R, kind="ExternalInput").ap()
    d["adjk"] = nc.dram_tensor("adjk", [2, FC], F32R, kind="ExternalInput").ap()
    d["adjv"] = nc.dram_tensor("adjv", [2, FC], F32R, kind="ExternalInput").ap()
    d["owh"] = nc.dram_tensor("owh", [HD, NH, D], BF16, kind="ExternalInput").ap()
    d["outb"] = nc.dram_tensor("outb", [D], F32, kind="ExternalInput").ap()
    d["xq"] = nc.dram_tensor("xq", [D, QS], F32, kind="ExternalInput").ap()
    d["w1p"] = nc.dram_tensor("w1p", [FF // P, P, KX, P], BF16,
                              kind="ExternalInput").ap()
    d["adjf"] = nc.dram_tensor("adjf", [2, FF], F32R, kind="ExternalInput").ap()
    d["w2T"] = nc.dram_tensor("w2T", [FF, D], BF16, kind="ExternalInput").ap()
    d["b2"] = nc.dram_tensor("b2", [D], F32, kind="ExternalInput").ap()
    d["out"] = nc.dram_tensor("out", [D, QS], F32, kind="ExternalOutput").ap()
    if os.environ.get("KERNEL_DEBUG_TAPS"):
        d["dbg_q"] = nc.dram_tensor("dbg_q", [P, FC // P, Q], BF16,
                                    kind="ExternalOutput").ap()
        d["dbg_k"] = nc.dram_tensor("dbg_k", [P, FC // P, S], BF16,
                                    kind="ExternalOutput").ap()
        d["dbg_v"] = nc.dram_tensor("dbg_v", [P, NST, NH, HD + 1], BF16,
                                    kind="ExternalOutput").ap()
        d["dbg_att"] = nc.dram_tensor("dbg_att", [NH, HD, Q], BF16,
                                      kind="ExternalOutput").ap()
        d["dbg_y1"] = nc.dram_tensor("dbg_y1", [P, KX, QS], F32,
                                     kind="ExternalOutput").ap()

    with tile.TileContext(nc) as tc:
        build_tile_program(tc, nc, d)
    nc.compile()   # bacc passes: wait splitting, ldweights fusion, reg alloc
    return nc


class _Pool:
    """Keeps the tile_pool context manager alive; allows explicit close."""

    def __init__(self, cm):
        self._cm = cm
        self.pool = cm.__enter__()

    def tile(self, *a, **kw):
        kw.setdefault("name", kw.get("tag") or "t")
        return self.pool.tile(*a, **kw)

    def close(self):
        self._cm.__exit__(None, None, None)


def build_tile_program(tc, nc, d):
    # ---------------- persistent constants ----------------
    const = _Pool(tc.tile_pool(name="const", bufs=1))
    dram = _Pool(tc.tile_pool(name="dram", bufs=1, space="DRAM"))

    ones_bf = const.tile([P, P], BF16, tag="ones_bf")
    nc.sync.dma_start(ones_bf, d["ones_bf"])
    ones_col = ones_bf[:, 0:1]                  # bf16 lhsT for column sums
    ident_bf = const.tile([P, P], BF16, tag="ident_bf")
    nc.sync.dma_start(ident_bf, d["ident_bf"])
    ones_row = ones_bf[0:1, :]                  # bf16 lhsT for broadcasts
    eps_t = const.tile([1, 1], F32, tag="eps")
    nc.vector.memset(eps_t, EPS)
    outb_col = const.tile([P, KX], F32, tag="outb_col")
    nc.sync.dma_start(outb_col, d["outb"].rearrange("(o p) -> p o", p=P))
    b2_col = const.tile([P, KX], F32, tag="b2_col")
    nc.sync.dma_start(b2_col, d["b2"].rearrange("(o p) -> p o", p=P))
    rz_col = const.tile([P, NST], F32, tag="rz_col")    # rstd_z token-striped

    rs_scr = dram.tile([1, S], F32, tag="rs_scr")       # row restripe bounce

    # ---------------- long-lived right-side pools ----------------
    pool_att = _Pool(tc.tile_pool(name="attp", bufs=1, side="right"))
    pool_qkv = _Pool(tc.tile_pool(name="qkv", bufs=1, side="right"))

    # ---------------- phase A/B scratch pools (left stack) ----------------
    pool_rows = _Pool(tc.tile_pool(name="rows", bufs=3))
    pool_sq = _Pool(tc.tile_pool(name="sq", bufs=3))
    pool_adj = _Pool(tc.tile_pool(name="adj", bufs=1))
    pool_z = _Pool(tc.tile_pool(name="pz", bufs=1))
    pool_x = _Pool(tc.tile_pool(name="px", bufs=1))

    # stat row tiles ([2, T] f32r lhsT/rhs operands for the rank-2 ADJ matmuls)
    adjx = pool_adj.tile([2, Q], F32R, tag="adjx")         # [mx ; stdx]
    adjz = pool_adj.tile([2, S], F32R, tag="adjz")         # [mz ; stdz]

    xT = pool_x.tile([P, KX, Q], BF16, tag="xT")
    for ch in range(2):
        for k in range(KX):
            nc.sync.dma_start(xT[:, k, ch * 512:(ch + 1) * 512],
                              d["xT"][k * P:(k + 1) * P, ch * 512:(ch + 1) * 512])
    zT = pool_z.tile([P, KX, S], BF16, tag="zT")
    for ch in range(4):
        for k in range(KX):
            nc.sync.dma_start(zT[:, k, ch * 512:(ch + 1) * 512],
                              d["zT"][k * P:(k + 1) * P, ch * 512:(ch + 1) * 512])

    def ln_stats_chunk(aT, sl, adj, rB, ps_stats, ps_bcp, scr=None):
        """One 512-token chunk: LN stats -> adj=[mean;std] rows, broadcast
        1/std into rB[:, sl]; optionally stash 1/std to scr (DRAM)."""
        ps_sum = ps_stats.tile([1, 512], F32, name="ps_sum", tag="ps_sum")
        ps_ssq = ps_stats.tile([1, 512], F32, name="ps_ssq", tag="ps_ssq")
        for k in range(KX):
            nc.tensor.matmul(ps_sum, ones_col, aT[:, k, sl],
                             start=(k == 0), stop=(k == KX - 1))
            sq = pool_sq.tile([P, 512], BF16, name="sq", tag="sq")
            nc.vector.tensor_mul(sq, aT[:, k, sl], aT[:, k, sl])
            nc.tensor.matmul(ps_ssq, ones_col, sq,
                             start=(k == 0), stop=(k == KX - 1))
        e2 = pool_rows.tile([1, 512], F32, name="e2", tag="e2")
        m2 = pool_rows.tile([1, 512], F32, name="m2", tag="m2")
        inv = pool_rows.tile([1, 512], F32R, name="inv", tag="inv")
        rr = pool_rows.tile([1, 512], F32, name="rr", tag="rr")
        rr_bf = pool_rows.tile([1, 512], BF16, name="rr_bf", tag="rr_bf")
        nc.vector.tensor_scalar_mul(adj[0:1, sl], ps_sum, 1.0 / D)  # mean
        nc.vector.tensor_scalar_mul(e2, ps_ssq, 1.0 / D)            # E[x^2]
        nc.vector.tensor_mul(m2, adj[0:1, sl], adj[0:1, sl])
        nc.vector.tensor_sub(e2, e2, m2)                            # var
        nc.scalar.activation(inv, e2, AF.Sqrt, bias=eps_t[0:1])     # std
        nc.vector.reciprocal_approx_fast(rr, inv.bitcast(F32))
        nc.sync.dma_start(adj[1:2, sl], inv)   # cross-partition row move
        nc.scalar.copy(rr_bf, rr)
        bc = ps_bcp.tile([P, 512], F32, name="bc", tag="bc")
        nc.tensor.matmul(bc, ones_row, rr_bf)
        nc.scalar.copy(rB[:, sl], bc)
        if scr is not None:
            nc.sync.dma_start(scr[0:1, sl], rr)

    # =================== x statistics + q projection (interleaved) =========
    qT = pool_qkv.tile([P, FC // P, Q], BF16, tag="qT")   # includes 1/8 scale
    kT = pool_qkv.tile([P, FC // P, S], BF16, tag="kT")
    v_sb = pool_qkv.tile([P, NST, NH, HD + 1], BF16, tag="v_sb")

    pool_bcx = _Pool(tc.tile_pool(name="bcx", bufs=1))
    rxB = pool_bcx.tile([P, Q], F32, tag="rxB")
    with tc.tile_pool(name="wq", bufs=1) as pool_wq, \
         tc.tile_pool(name="ps_sx", bufs=1, space="PSUM") as ps_sx, \
         tc.tile_pool(name="ps_bcx", bufs=1, space="PSUM") as ps_bcx, \
         tc.tile_pool(name="ps_q", bufs=4, space="PSUM") as ps_qk:
        wq_sb = pool_wq.tile([P, KX, FC], BF16, tag="wq_sb")
        for k in range(KX):
            nc.sync.dma_start(wq_sb[:, k, :], d["wqT"][:, k, :])
        adjq_w = pool_wq.tile([2, FC], F32R, tag="adjq_w")
        nc.sync.dma_start(adjq_w, d["adjq"])

        # raw matmuls run a chunk ahead of the adj+scale finish (the LN-stats
        # chain has a full chunk of PE work to hide under)
        pend_q = []

        def finish_q(ch, pss):
            sl = slice(ch * 512, (ch + 1) * 512)
            for m in range(FC // P):
                nc.tensor.matmul(pss[m], adjq_w[:, m * P:(m + 1) * P],
                                 adjx[:, sl], start=False, stop=True)
                nc.vector.tensor_mul(qT[:, m, sl], pss[m], rxB[:, sl])

        for ch in range(Q // 512):
            sl = slice(ch * 512, (ch + 1) * 512)
            ln_stats_chunk(xT, sl, adjx, rxB, ps_sx, ps_bcx)
            pss = []
            for m in range(FC // P):
                ps = ps_qk.tile([P, 512], F32, name="ps_qk_t", tag="ps_qk_t")
                for k in range(KX):
                    nc.tensor.matmul(ps, wq_sb[:, k, m * P:(m + 1) * P],
                                     xT[:, k, sl], start=(k == 0), stop=False)
                pss.append(ps)
            pend_q.append((ch, pss))
            if len(pend_q) > 1:
                finish_q(*pend_q.pop(0))
        while pend_q:
            finish_q(*pend_q.pop(0))

    pool_bcx.close()
    pool_x.close()

    # =================== z statistics + k/v projections (interleaved) ======
    # softmax-denominator ones column
    nc.sync.dma_start(
        v_sb[:, :, :, HD:HD + 1],
        d["ones_bf"][:, 0:NST * NH].rearrange("p (a b c) -> p a b c",
                                              a=NST, c=1))

    pool_bcz = _Pool(tc.tile_pool(name="bcz", bufs=1))
    rzB = pool_bcz.tile([P, S], F32, tag="rzB")
    with tc.tile_pool(name="wkv", bufs=1) as pool_wkv, \
         tc.tile_pool(name="ps_sz", bufs=1, space="PSUM") as ps_sz, \
         tc.tile_pool(name="ps_bcz", bufs=1, space="PSUM") as ps_bcz, \
         tc.tile_pool(name="ps_k", bufs=2, space="PSUM") as ps_k, \
         tc.tile_pool(name="ps_v", bufs=2, space="PSUM") as ps_v:
        wk_sb = pool_wkv.tile([P, KX, FC], BF16, tag="wk_sb")
        wv_sb = pool_wkv.tile([P, KX, FC], BF16, tag="wv_sb")
        for k in range(KX):
            nc.sync.dma_start(wk_sb[:, k, :], d["wkT"][:, k, :])
            nc.sync.dma_start(wv_sb[:, k, :], d["wvT"][:, k, :])
        adjk_w = pool_wkv.tile([2, FC], F32R, tag="adjk_w")
        nc.sync.dma_start(adjk_w, d["adjk"])
        adjv_w = pool_wkv.tile([2, FC], F32R, tag="adjv_w")
        nc.sync.dma_start(adjv_w, d["adjv"])

        def v_raw(t):
            ps = ps_v.tile([P, FC], F32, name="ps_v_t", tag="ps_v_t")
            for k in range(KX):
                nc.tensor.matmul(ps, zT[:, k, t * P:(t + 1) * P],
                                 wv_sb[:, k, :], start=(k == 0), stop=False)
            return ps

        def v_fin(t, ps):
            nc.tensor.matmul(ps, adjz[:, t * P:(t + 1) * P],
                             adjv_w, start=False, stop=True)
            nc.vector.tensor_scalar_mul(
                v_sb[:, t, :, 0:HD],
                ps.rearrange("p (h e) -> p h e", h=NH),
                rz_col[:, t:t + 1])

        for ch in range(S // 512):
            sl = slice(ch * 512, (ch + 1) * 512)
            ln_stats_chunk(zT, sl, adjz, rzB, ps_sz, ps_bcz, scr=rs_scr)
            # restripe this chunk's 1/std row into token-striped columns
            nc.sync.dma_start(
                rz_col[:, ch * 4:(ch + 1) * 4],
                rs_scr[0:1, sl].rearrange("a (i p) -> (a p) i", p=P))
            # raw k/v matmuls cover the stats chain; adj+scale finish after
            pss = []
            for m in range(FC // P):
                ps = ps_k.tile([P, 512], F32, name="ps_k_t", tag="ps_k_t")
                for k in range(KX):
                    nc.tensor.matmul(ps, wk_sb[:, k, m * P:(m + 1) * P],
                                     zT[:, k, sl], start=(k == 0), stop=False)
                pss.append(ps)
            vps = [v_raw(ch * 4 + ti) for ti in range(2)]
            for m in range(FC // P):
                nc.tensor.matmul(pss[m], adjk_w[:, m * P:(m + 1) * P],
                                 adjz[:, sl], start=False, stop=True)
                nc.vector.tensor_mul(kT[:, m, sl], pss[m], rzB[:, sl])
            for ti in range(2):
                v_fin(ch * 4 + ti, vps[ti])
            vps = [v_raw(ch * 4 + ti) for ti in range(2, 4)]
            for ti in range(2):
                v_fin(ch * 4 + 2 + ti, vps[ti])

    pool_bcz.close()
    pool_z.close()
    pool_adj.close()
    pool_sq.close()
    pool_rows.close()

    # =================== attention ===================
    att64 = [pool_att.tile([HD, Q], BF16, name=f"att64_{h}", tag=f"att64_{h}")
             for h in range(NH)]

    with tc.tile_pool(name="mask", bufs=20) as pool_mask, \
         tc.tile_pool(name="probs", bufs=6) as pool_probs, \
         tc.tile_pool(name="rrow", bufs=2) as pool_rrow, \
         tc.tile_pool(name="ps_lg", bufs=2, space="PSUM") as ps_lg, \
         tc.tile_pool(name="ps_att", bufs=1, space="PSUM") as ps_att, \
         tc.tile_pool(name="ps_nbc", bufs=1, space="PSUM") as ps_nbc:

        steps = [(h, st) for h in range(NH) for st in range(NST)]
        att_ps = [None]         # current head's PSUM accumulator (list for closure)
        prs = {}                # g -> probs tile

        def emit_front(g):
            h, st = steps[g]
            mk = pool_mask.tile([P, Q], BF16, name="mk", tag="mk")
            nc.sync.dma_start(mk, d["maskT"][h, st * P:(st + 1) * P, :])
            lg = ps_lg.tile([P, Q], F32, name="lg", tag="lg")
            for ch in range(Q // 512):
                sl = slice(ch * 512, (ch + 1) * 512)
                ht, ho = h // 2, HD * (h % 2)
                nc.tensor.matmul(lg[:, sl],
                                 kT[ho:ho + HD, ht, st * P:(st + 1) * P],
                                 qT[ho:ho + HD, ht, sl],
                                 start=True, stop=False)
                nc.tensor.matmul(lg[:, sl], ident_bf, mk[:, sl],
                                 start=False, stop=True)
            pr = pool_probs.tile([P, Q], BF16, name="pr", tag="pr")
            nc.scalar.activation(pr, lg, AF.Exp)
            prs[g] = pr

        def emit_back(g):
            h, st = steps[g]
            if st == 0:
                att_ps[0] = ps_att.tile([HD + 1, Q], F32, name="att_ps",
                                        tag="att_ps")
            pr = prs.pop(g)
            for ch in range(Q // 512):
                sl = slice(ch * 512, (ch + 1) * 512)
                nc.tensor.matmul(att_ps[0][:, sl], v_sb[:, st, h, :],
                                 pr[:, sl], start=(st == 0),
                                 stop=(st == NST - 1))
            if st == NST - 1:
                # normalize: att[0:64] * broadcast(1 / att[64]).  The
                # denominator row must bounce through SBUF: the approx
                # reciprocal (custom DVE op) misreads PSUM operands on HW.
                ap = att_ps[0]
                den = pool_rrow.tile([1, Q], F32, name="den", tag="den")
                nc.vector.tensor_copy(den, ap[HD:HD + 1, :])
                rr = pool_rrow.tile([1, Q], F32, name="rr2", tag="rr2")
                nc.vector.reciprocal_approx_fast(rr, den)
                rr_bf = pool_rrow.tile([1, Q], BF16, name="rrb2", tag="rrb2")
                nc.scalar.copy(rr_bf, rr)
                nbc = ps_nbc.tile([HD, Q], F32, name="nbc", tag="nbc")
                nbs = pool_rrow.tile([HD, Q], F32, name="nbs", tag="nbs")
                for ch in range(Q // 512):
                    sl = slice(ch * 512, (ch + 1) * 512)
                    nc.tensor.matmul(nbc[:, sl], ones_bf[0:1, 0:HD],
                                     rr_bf[:, sl])
                nc.scalar.copy(nbs, nbc)
                nc.vector.tensor_mul(att64[h], ap[0:HD, :], nbs)

        DEPTH = 2
        for g in range(len(steps) + DEPTH):
            if g < len(steps):
                emit_front(g)
            if g - DEPTH >= 0:
                emit_back(g - DEPTH)

    if "dbg_q" in d:
        nc.sync.dma_start(d["dbg_q"], qT)
        nc.sync.dma_start(d["dbg_k"], kT)
        nc.sync.dma_start(d["dbg_v"], v_sb)
        for h in range(NH):
            nc.sync.dma_start(d["dbg_att"][h], att64[h])

    pool_qkv.close()

    # =================== out-projection + pipelined ReduceScatter ==========
    MC = KX // RSCH            # m-tiles per RS chunk (2)
    rs_in = [dram.tile([4, MC * P, QS], BF16, name=f"rs_in{c}", tag=f"rs_in{c}")
             for c in range(RSCH)]
    rs_out = [dram.tile([MC * P, QS], BF16, name=f"rs_out{c}", tag=f"rs_out{c}")
              for c in range(RSCH)]

    with tc.tile_pool(name="ow", bufs=1) as pool_ow, \
         tc.tile_pool(name="osb", bufs=3) as pool_osb, \
         tc.tile_pool(name="ps_o", bufs=3, space="PSUM") as ps_o:
        ow_sb = pool_ow.tile([HD, NH, D], BF16, tag="ow_sb")
        for h in range(NH):
            nc.sync.dma_start(ow_sb[:, h, :], d["owh"][:, h, :])
        for c in range(RSCH):
            for mi in range(MC):
                m = c * MC + mi
                for ch in range(Q // 512):
                    sl = slice(ch * 512, (ch + 1) * 512)
                    ps = ps_o.tile([P, 512], F32, name="ps_o_t", tag="ps_o_t")
                    for h in range(NH):
                        nc.tensor.matmul(ps, ow_sb[:, h, m * P:(m + 1) * P],
                                         att64[h][:, sl],
                                         start=(h == 0), stop=(h == NH - 1))
                    ot = pool_osb.tile([P, 512], BF16, name="ot", tag="ot")
                    nc.scalar.copy(ot, ps)
                    for r2 in range(2):
                        nc.sync.dma_start(
                            rs_in[c][2 * ch + r2, mi * P:(mi + 1) * P, :],
                            ot[:, r2 * QS:(r2 + 1) * QS])
            nc.gpsimd.collective_compute(
                "ReduceScatter",
                ALU.add,
                replica_groups=REPLICA_GROUPS,
                ins=[rs_in[c].opt()],
                outs=[rs_out[c].opt()],
            )

    pool_att.close()

    # =================== residual + FFN (sequence-parallel) ===================
    with tc.tile_pool(name="ffn", bufs=1) as pool_f, \
         tc.tile_pool(name="w1s", bufs=16) as pool_w1, \
         tc.tile_pool(name="w2s", bufs=8) as pool_w2, \
         tc.tile_pool(name="gact", bufs=3) as pool_g, \
         tc.tile_pool(name="rsld", bufs=4) as pool_rsld, \
         tc.tile_pool(name="yout", bufs=3) as pool_yo:

        y1T = pool_f.tile([P, KX, QS], F32, tag="y1T")
        y1b = pool_f.tile([P, KX, QS], BF16, tag="y1b")
        adjy = pool_f.tile([2, QS], F32R, tag="adjy")     # [my ; stdy]
        ry_row = pool_f.tile([1, QS], F32, tag="ry_row")
        ry_bf = pool_f.tile([1, QS], BF16, tag="ry_bf")
        ryB = pool_f.tile([P, QS], F32, tag="ryB")
        adjf_w = pool_f.tile([2, FF], F32R, tag="adjf_w")
        nc.sync.dma_start(adjf_w, d["adjf"])

        # y1 = RS(out-proj partials) + x_slice + out_b   (feature-major)
        # gpsimd DMAs: keep the HWDGE queues free while collectives drain
        for c in range(RSCH):
            for ki in range(MC):
                k = c * MC + ki
                rst = pool_rsld.tile([P, QS], BF16, name="rst", tag="rst")
                nc.gpsimd.dma_start(rst, rs_out[c][ki * P:(ki + 1) * P, :])
                xqt = pool_rsld.tile([P, QS], F32, name="xqt", tag="xqt")
                nc.gpsimd.dma_start(xqt, d["xq"][k * P:(k + 1) * P, :])
                nc.vector.scalar_tensor_tensor(
                    out=y1T[:, k, :], in0=rst, scalar=outb_col[:, k:k + 1],
                    in1=xqt, op0=ALU.add, op1=ALU.add)
                nc.scalar.copy(y1b[:, k, :], y1T[:, k, :])
        if "dbg_y1" in d:
            nc.sync.dma_start(d["dbg_y1"], y1T)

        # y1 LN stats
        with tc.tile_pool(name="ps_yst", bufs=1, space="PSUM") as ps_yst:
            e2_row = pool_f.tile([1, QS], F32, tag="e2y_row")
            m2_row = pool_f.tile([1, QS], F32, tag="m2y_row")
            inv_row = pool_f.tile([1, QS], F32R, tag="invy_row")
            ps_sum = ps_yst.tile([1, QS], F32, name="ps_sum2", tag="ps_sum2")
            ps_ssq = ps_yst.tile([1, QS], F32, name="ps_ssq2", tag="ps_ssq2")
            for k in range(KX):
                nc.tensor.matmul(ps_sum, ones_col, y1b[:, k, :],
                                 start=(k == 0), stop=(k == KX - 1))
                sq = pool_g.tile([P, QS], BF16, name="ysq", tag="gt")
                nc.scalar.square(sq, y1b[:, k, :])
                nc.tensor.matmul(ps_ssq, ones_col, sq,
                                 start=(k == 0), stop=(k == KX - 1))
            nc.vector.tensor_scalar_mul(adjy[0:1, :], ps_sum, 1.0 / D)
            nc.vector.tensor_scalar_mul(e2_row, ps_ssq, 1.0 / D)
            nc.vector.tensor_mul(m2_row, adjy[0:1, :], adjy[0:1, :])
            nc.vector.tensor_sub(e2_row, e2_row, m2_row)
            nc.scalar.activation(inv_row, e2_row, AF.Sqrt, bias=eps_t[0:1])
            nc.vector.reciprocal_approx_fast(ry_row, inv_row.bitcast(F32))
            nc.sync.dma_start(adjy[1:2, :], inv_row)
            nc.scalar.copy(ry_bf, ry_row)
            bc = ps_yst.tile([P, QS], F32, name="bc3", tag="bc3")
            nc.tensor.matmul(bc, ones_row, ry_bf)
            nc.scalar.copy(ryB, bc)

        # ff1 (gelu) into a persistent activation tile, software-pipelined
        # with ff2's first half-D pass; second half-D pass afterwards.
        g_sb = pool_f.tile([P, FF // P, QS], BF16, tag="g_sb")
        DH = D // 2
        ps_ff = tc.tile_pool(name="ps_ff", bufs=2, space="PSUM")
        ps_f = ps_ff.__enter__()
        ps_y2cm = tc.tile_pool(name="ps_y2", bufs=1, space="PSUM")
        ps_y2 = ps_y2cm.__enter__()

        def alloc_y2():
            return [ps_y2.tile([P, QS], F32, name=f"y2a_{i}", tag=f"y2a_{i}",
                               bufs=1) for i in range(4)]

        def emit_ff1(j):
            w1b = pool_w1.tile([P, KX, P], BF16, name="w1b", tag="w1b")
            nc.sync.dma_start(w1b, d["w1p"][j])
            ps = ps_f.tile([P, QS], F32, name="ps_f_t", tag="ps_f_t")
            for k in range(KX):
                nc.tensor.matmul(ps, w1b[:, k, :], y1b[:, k, :],
                                 start=(k == 0), stop=False)
            nc.tensor.matmul(ps, adjf_w[:, j * P:(j + 1) * P], adjy,
                             start=False, stop=True)
            nc.vector.tensor_mul(g_sb[:, j, :], ps, ryB)  # ff1 = ry*(raw+adj)
            gelu_af = (AF.Sigmoid if os.environ.get("SIM_DEBUG_SIGMOID")
                       else AF.Gelu)
            nc.scalar.activation(g_sb[:, j, :], g_sb[:, j, :], gelu_af)

        def emit_ff2(y2a, half, j):
            w2b = pool_w2.tile([P, DH], BF16, name="w2b", tag="w2b")
            nc.sync.dma_start(
                w2b, d["w2T"][j * P:(j + 1) * P, half * DH:(half + 1) * DH])
            for mi in range(4):
                nc.tensor.matmul(y2a[mi], w2b[:, mi * P:(mi + 1) * P],
                                 g_sb[:, j, :],
                                 start=(j == 0), stop=(j == FF // P - 1))

        def emit_y2_out(y2a, half):
            for mi in range(4):
                m = half * 4 + mi
                yt = pool_yo.tile([P, QS], F32, name="yt", tag="yt")
                nc.vector.scalar_tensor_tensor(
                    out=yt, in0=y2a[mi], scalar=b2_col[:, m:m + 1],
                    in1=y1T[:, m, :], op0=ALU.add, op1=ALU.add)
                nc.sync.dma_start(d["out"][m * P:(m + 1) * P, :], yt)

        y2h0 = alloc_y2()
        for j in range(FF // P + 1):
            if j < FF // P:
                emit_ff1(j)
            if j - 1 >= 0:
                emit_ff2(y2h0, 0, j - 1)
        emit_y2_out(y2h0, 0)
        y2h1 = alloc_y2()
        for j in range(FF // P):
            emit_ff2(y2h1, 1, j)
        emit_y2_out(y2h1, 1)
        ps_y2cm.__exit__(None, None, None)
        ps_ff.__exit__(None, None, None)

    const.close()
    dram.close()


def host_prep(inputs):
    """Fold layernorm gains/biases into weights; build the 8 per-core shards."""
    f32 = np.float32
    x = np.asarray(inputs["x"], f32)
    z = np.asarray(inputs["z"], f32)
    mask = np.asarray(inputs["attn_mask"], f32)
    gq = np.asarray(inputs["gq"], np.float64)
    bq = np.asarray(inputs["bq"], np.float64)
    gkv = np.asarray(inputs["gkv"], np.float64)
    bkv = np.asarray(inputs["bkv"], np.float64)
    gff = np.asarray(inputs["gff"], np.float64)
    bff = np.asarray(inputs["bff"], np.float64)
    ipw = np.asarray(inputs["in_proj_w"], np.float64)
    ipb = np.asarray(inputs["in_proj_b"], np.float64)
    out_w = np.asarray(inputs["out_w"], f32)
    out_b = np.asarray(inputs["out_b"], f32)
    w1 = np.asarray(inputs["w1"], np.float64)
    b1 = np.asarray(inputs["b1"], np.float64)
    w2 = np.asarray(inputs["w2"], f32)
    b2 = np.asarray(inputs["b2"], f32)

    wq, wk, wv = ipw[:D], ipw[D:2 * D], ipw[2 * D:]
    pq, pk, pv = ipb[:D], ipb[D:2 * D], ipb[2 * D:]
    scale = 1.0 / np.sqrt(HD)
    wq2 = (wq * gq[None, :]) * scale
    pq2 = (wq @ bq + pq) * scale
    wk2 = wk * gkv[None, :]
    pk2 = wk @ bkv + pk
    wv2 = wv * gkv[None, :]
    pv2 = wv @ bkv + pv
    w12 = w1 * gff[None, :]
    b12 = w1 @ bff + b1

    w1T = np.ascontiguousarray(w12.T.astype(f32))                    # (D, FF)
    # packed so each hidden-block's [P, KX, P] lhsT tile set is contiguous
    w1p = np.ascontiguousarray(
        w1T.reshape(KX, P, FF // P, P).transpose(2, 1, 0, 3).astype(BF))
    adjf = np.ascontiguousarray(
        np.stack([-w12.sum(1), b12]).astype(f32))                    # (2, FF)
    w2T = np.ascontiguousarray(w2.T.astype(BF))                      # (FF, D)

    def pack_kxf(wT):  # (D, FC) -> (P, D//P, FC) in bf16
        return np.ascontiguousarray(
            wT.reshape(KX, P, FC).transpose(1, 0, 2).astype(BF))

    ident = np.eye(P, dtype=BF)

    in_maps = []
    for c in range(NCORES):
        b, hg = c // 4, c % 4
        fs = slice(FC * hg, FC * hg + FC)
        qs = slice(QS * (c % 4), QS * (c % 4) + QS)
        xTb = np.ascontiguousarray(x[b].T)                           # (D, Q)
        in_maps.append({
            "ones_bf": np.ones((P, P), BF),
            "ident_bf": ident,
            "xT": xTb.astype(BF),
            "zT": np.ascontiguousarray(z[b].T).astype(BF),
            "maskT": np.ascontiguousarray(
                mask[16 * b + NH * hg:16 * b + NH * hg + NH]
                .transpose(0, 2, 1)).astype(BF),
            "wqT": pack_kxf(np.ascontiguousarray(wq2[fs].T.astype(f32))),
            "wkT": pack_kxf(np.ascontiguousarray(wk2[fs].T.astype(f32))),
            "wvT": pack_kxf(np.ascontiguousarray(wv2[fs].T.astype(f32))),
            "adjq": np.ascontiguousarray(
                np.stack([-wq2[fs].sum(1), pq2[fs]]).astype(f32)),
            "adjk": np.ascontiguousarray(
                np.stack([-wk2[fs].sum(1), pk2[fs]]).astype(f32)),
            "adjv": np.ascontiguousarray(
                np.stack([-wv2[fs].sum(1), pv2[fs]]).astype(f32)),
            "owh": np.ascontiguousarray(
                out_w[:, fs].T.reshape(NH, HD, D).transpose(1, 0, 2)
                .astype(BF)),
            "outb": out_b,
            "xq": np.ascontiguousarray(xTb[:, qs]),
            "w1p": w1p,
            "adjf": adjf,
            "w2T": w2T,
            "b2": b2,
        })
    return in_maps


_NC_CACHE = None


def kernel(**inputs) -> np.ndarray:
    global _NC_CACHE, LAST_RESULT
    from concourse.bass_utils import run_bass_kernel_spmd

    in_maps = host_prep(inputs)
    if _NC_CACHE is None:
        _NC_CACHE = build_nc()
    res = run_bass_kernel_spmd(
        _NC_CACHE, in_maps, core_ids=list(range(NCORES)),
        trace=bool(os.environ.get("BASS_TRACE")),
    )
    LAST_RESULT = res
    out = np.empty((B, Q, D), np.float32)
    for c in range(NCORES):
        b = c // 4
        qs = slice(QS * (c % 4), QS * (c % 4) + QS)
        out[b, qs, :] = res.results[c]["out"].T
    return out


# revision 50
# speedup vs baseline: 1.2651x; 1.0691x over previous
# Bass/Tile TRN2 kernel for nn_BiasedCrossDecoderLayer (dense cross-attention
# transformer decoder layer), SPMD over 8 NeuronCores.
#
# Sharding: core c -> batch b = c//4, head-group hg = c%4 (4 of 16 heads =
# 256 of 1024 qkv feature dims).  Attention is head-parallel; the
# out-projection produces partial sums which are ReduceScattered (along the
# query axis, in 4 pipelined bf16 chunks) within each 4-core batch group;
# the FFN then runs sequence-parallel on each core's 256-query slice.
#
# All matmul operands are bf16 (fp32 PSUM accumulation): on TRN2 hardware
# fp32r streams at ~1.6-2 cycles/row while bf16 streams at 1.0, and bf16
# halves mask/weight HBM traffic.  LayerNorms are folded into the weights
# host-side (see ADJ rank-2 matmul trick below); the adj matmuls stay fp32r.
#
#   q = LN(x;g,b) @ Wq.T + pq  ==  LN0(x) @ Wq'.T + bias'
#   qT = rB * (Wq' @ xT_raw + ADJ)
#   ADJ[o,t] = -rowsum(Wq')[o]*m[t] + bias'[o]*std[t]    (rank-2 matmul
#              appended to the same PSUM accumulation group)
#
# Attention runs in the transposed [s, q] layout (mask pre-transposed on the
# host).  The softmax denominator comes from a ones-column appended to the V
# stationary operand (M=65 matmul); 1/sum via reciprocal_approx_fast.  The
# mask+logits add is split between the DVE and PE (identity-matmul accumulate
# into the logits PSUM group) to balance engines; the (logits -> add -> exp ->
# PV) chain is software-pipelined by 2 steps on the in-order PE queue.

import os
import sys

import numpy as np

sys.path.insert(0, "/opt/trn_rl_repo")

import ml_dtypes  # noqa: E402

import concourse.bass as bass  # noqa: E402
import concourse.mybir as mybir  # noqa: E402
import concourse.tile as tile  # noqa: E402
from concourse import bacc  # noqa: E402

F32 = mybir.dt.float32
F32R = mybir.dt.float32r
BF16 = mybir.dt.bfloat16
AF = mybir.ActivationFunctionType
ALU = mybir.AluOpType
BF = ml_dtypes.bfloat16

B, Q, S, D, H = 2, 1024, 2048, 1024, 16
HD = D // H       # 64
FF = 4 * D
EPS = 1e-5
NCORES = 8
NH = 4            # heads per core
FC = NH * HD      # 256 qkv feature dims per core
QS = Q // 4       # 256-query slice per core after reduce-scatter
P = 128
KX = D // P       # 8 k-tiles over the model dim
NST = S // P      # 16 s-tiles
RSCH = 2          # reduce-scatter chunks (collective overhead is ~10us each)

REPLICA_GROUPS = [[0, 1, 2, 3], [4, 5, 6, 7]]

LAST_RESULT = None  # BassKernelResults of the most recent run (for test.py)


def _r(ap):
    """View an fp32 AP as float32r for full-rate PE matmuls."""
    return ap.bitcast(F32R)


def build_nc():
    nc = bacc.Bacc(
        "TRN2",
        target_bir_lowering=False,
        debug=False,
        num_devices=NCORES,
        name="biased_cross_decoder",
    )

    # ---- DRAM I/O (per-core shards; same program on all cores) ----
    d = {}
    d["ones_bf"] = nc.dram_tensor("ones_bf", [P, P], BF16, kind="ExternalInput").ap()
    d["ident_bf"] = nc.dram_tensor("ident_bf", [P, P], BF16, kind="ExternalInput").ap()
    d["xT"] = nc.dram_tensor("xT", [D, Q], BF16, kind="ExternalInput").ap()
    d["zT"] = nc.dram_tensor("zT", [D, S], BF16, kind="ExternalInput").ap()
    d["maskT"] = nc.dram_tensor("maskT", [NH, S, Q], BF16, kind="ExternalInput").ap()
    d["wqT"] = nc.dram_tensor("wqT", [P, KX, FC], BF16, kind="ExternalInput").ap()
    d["wkT"] = nc.dram_tensor("wkT", [P, KX, FC], BF16, kind="ExternalInput").ap()
    d["wvT"] = nc.dram_tensor("wvT", [P, KX, FC], BF16, kind="ExternalInput").ap()
    d["adjq"] = nc.dram_tensor("adjq", [2, FC], F32R, kind="ExternalInput").ap()
    d["adjk"] = nc.dram_tensor("adjk", [2, FC], F32R, kind="ExternalInput").ap()
    d["adjv"] = nc.dram_tensor("adjv", [2, FC], F32R, kind="ExternalInput").ap()
    d["owh"] = nc.dram_tensor("owh", [HD, NH, D], BF16, kind="ExternalInput").ap()
    d["outb"] = nc.dram_tensor("outb", [D], F32, kind="ExternalInput").ap()
    d["xq"] = nc.dram_tensor("xq", [D, QS], F32, kind="ExternalInput").ap()
    d["w1p"] = nc.dram_tensor("w1p", [FF // P, P, KX, P], BF16,
                              kind="ExternalInput").ap()
    d["adjf"] = nc.dram_tensor("adjf", [2, FF], F32R, kind="ExternalInput").ap()
    d["w2T"] = nc.dram_tensor("w2T", [FF, D], BF16, kind="ExternalInput").ap()
    d["b2"] = nc.dram_tensor("b2", [D], F32, kind="ExternalInput").ap()
    d["out"] = nc.dram_tensor("out", [D, QS], F32, kind="ExternalOutput").ap()
    if os.environ.get("KERNEL_DEBUG_TAPS"):
        d["dbg_q"] = nc.dram_tensor("dbg_q", [P, FC // P, Q], BF16,
                                    kind="ExternalOutput").ap()
        d["dbg_k"] = nc.dram_tensor("dbg_k", [P, FC // P, S], BF16,
                                    kind="ExternalOutput").ap()
        d["dbg_v"] = nc.dram_tensor("dbg_v", [P, NST, NH, HD + 1], BF16,
                                    kind="ExternalOutput").ap()
        d["dbg_att"] = nc.dram_tensor("dbg_att", [NH, HD, Q], BF16,
                                      kind="ExternalOutput").ap()
        d["dbg_y1"] = nc.dram_tensor("dbg_y1", [P, KX, QS], F32,
                                     kind="ExternalOutput").ap()

    with tile.TileContext(nc) as tc:
        build_tile_program(tc, nc, d)
    nc.compile()   # bacc passes: wait splitting, ldweights fusion, reg alloc
    return nc


class _Pool:
    """Keeps the tile_pool context manager alive; allows explicit close."""

    def __init__(self, cm):
        self._cm = cm
        self.pool = cm.__enter__()

    def tile(self, *a, **kw):
        kw.setdefault("name", kw.get("tag") or "t")
        return self.pool.tile(*a, **kw)

    def close(self):
        self._cm.__exit__(None, None, None)


def build_tile_program(tc, nc, d):
    # ---------------- persistent constants ----------------
    const = _Pool(tc.tile_pool(name="const", bufs=1))
    dram = _Pool(tc.tile_pool(name="dram", bufs=1, space="DRAM"))

    ones_bf = const.tile([P, P], BF16, tag="ones_bf")
    nc.sync.dma_start(ones_bf, d["ones_bf"])
    ones_col = ones_bf[:, 0:1]                  # bf16 lhsT for column sums
    ident_bf = const.tile([P, P], BF16, tag="ident_bf")
    nc.sync.dma_start(ident_bf, d["ident_bf"])
    ones_row = ones_bf[0:1, :]                  # bf16 lhsT for broadcasts
    eps_t = const.tile([1, 1], F32, tag="eps")
    nc.vector.memset(eps_t, EPS)
    outb_col = const.tile([P, KX], F32, tag="outb_col")
    nc.sync.dma_start(outb_col, d["outb"].rearrange("(o p) -> p o", p=P))
    b2_col = const.tile([P, KX], F32, tag="b2_col")
    nc.sync.dma_start(b2_col, d["b2"].rearrange("(o p) -> p o", p=P))
    rz_col = const.tile([P, NST], F32, tag="rz_col")    # rstd_z token-striped

    rs_scr = dram.tile([1, S], F32, tag="rs_scr")       # row restripe bounce

    # ---------------- long-lived right-side pools ----------------
    pool_att = _Pool(tc.tile_pool(name="attp", bufs=1, side="right"))
    pool_qkv = _Pool(tc.tile_pool(name="qkv", bufs=1, side="right"))

    # ---------------- phase A/B scratch pools (left stack) ----------------
    pool_rows = _Pool(tc.tile_pool(name="rows", bufs=3))
    pool_sq = _Pool(tc.tile_pool(name="sq", bufs=3))
    pool_adj = _Pool(tc.tile_pool(name="adj", bufs=1))
    pool_z = _Pool(tc.tile_pool(name="pz", bufs=1))
    pool_x = _Pool(tc.tile_pool(name="px", bufs=1))

    # stat row tiles ([2, T] f32r lhsT/rhs operands for the rank-2 ADJ matmuls)
    adjx = pool_adj.tile([2, Q], F32R, tag="adjx")         # [mx ; stdx]
    adjz = pool_adj.tile([2, S], F32R, tag="adjz")         # [mz ; stdz]

    xT = pool_x.tile([P, KX, Q], BF16, tag="xT")
    for ch in range(2):
        for k in range(KX):
            nc.sync.dma_start(xT[:, k, ch * 512:(ch + 1) * 512],
                              d["xT"][k * P:(k + 1) * P, ch * 512:(ch + 1) * 512])
    zT = pool_z.tile([P, KX, S], BF16, tag="zT")
    for ch in range(4):
        for k in range(KX):
            nc.sync.dma_start(zT[:, k, ch * 512:(ch + 1) * 512],
                              d["zT"][k * P:(k + 1) * P, ch * 512:(ch + 1) * 512])

    def ln_stats_chunk(aT, sl, adj, ps_stats, scr=None):
        """One 512-token chunk: LN stats -> adj=[mean;std] rows + bf16 1/std
        row; the PE broadcast of 1/std is deferred (see broadcast_rstd) so it
        doesn't head-of-line-block the PE queue behind the DVE/Act chain."""
        ps_sum = ps_stats.tile([1, 512], F32, name="ps_sum", tag="ps_sum")
        ps_ssq = ps_stats.tile([1, 512], F32, name="ps_ssq", tag="ps_ssq")
        for k in range(KX):
            nc.tensor.matmul(ps_sum, ones_col, aT[:, k, sl],
                             start=(k == 0), stop=(k == KX - 1))
            sq = pool_sq.tile([P, 512], BF16, name="sq", tag="sq")
            nc.vector.tensor_mul(sq, aT[:, k, sl], aT[:, k, sl])
            nc.tensor.matmul(ps_ssq, ones_col, sq,
                             start=(k == 0), stop=(k == KX - 1))
        e2 = pool_rows.tile([1, 512], F32, name="e2", tag="e2")
        m2 = pool_rows.tile([1, 512], F32, name="m2", tag="m2")
        inv = pool_rows.tile([1, 512], F32R, name="inv", tag="inv")
        rr = pool_rows.tile([1, 512], F32, name="rr", tag="rr")
        rr_bf = pool_rows.tile([1, 512], BF16, name="rr_bf", tag="rr_bf")
        nc.vector.tensor_scalar_mul(adj[0:1, sl], ps_sum, 1.0 / D)  # mean
        nc.vector.tensor_scalar_mul(e2, ps_ssq, 1.0 / D)            # E[x^2]
        nc.vector.tensor_mul(m2, adj[0:1, sl], adj[0:1, sl])
        nc.vector.tensor_sub(e2, e2, m2)                            # var
        nc.scalar.activation(inv, e2, AF.Sqrt, bias=eps_t[0:1])     # std
        nc.vector.reciprocal_approx_fast(rr, inv.bitcast(F32))
        nc.sync.dma_start(adj[1:2, sl], inv)   # cross-partition row move
        nc.scalar.copy(rr_bf, rr)
        if scr is not None:
            nc.sync.dma_start(scr[0:1, sl], rr)
        return rr_bf

    def broadcast_rstd(rr_bf, rB, sl, ps_bcp):
        bc = ps_bcp.tile([P, 512], F32, name="bc", tag="bc")
        nc.tensor.matmul(bc, ones_row, rr_bf)
        nc.scalar.copy(rB[:, sl], bc)

    # =================== x statistics + q projection (interleaved) =========
    qT = pool_qkv.tile([P, FC // P, Q], BF16, tag="qT")   # includes 1/8 scale
    kT = pool_qkv.tile([P, FC // P, S], BF16, tag="kT")
    v_sb = pool_qkv.tile([P, NST, NH, HD + 1], BF16, tag="v_sb")

    pool_bcx = _Pool(tc.tile_pool(name="bcx", bufs=1))
    rxB = pool_bcx.tile([P, Q], F32, tag="rxB")
    with tc.tile_pool(name="wq", bufs=1) as pool_wq, \
         tc.tile_pool(name="ps_sx", bufs=1, space="PSUM") as ps_sx, \
         tc.tile_pool(name="ps_bcx", bufs=1, space="PSUM") as ps_bcx, \
         tc.tile_pool(name="ps_q", bufs=4, space="PSUM") as ps_qk:
        wq_sb = pool_wq.tile([P, KX, FC], BF16, tag="wq_sb")
        for k in range(KX):
            nc.sync.dma_start(wq_sb[:, k, :], d["wqT"][:, k, :])
        adjq_w = pool_wq.tile([2, FC], F32R, tag="adjq_w")
        nc.sync.dma_start(adjq_w, d["adjq"])

        # raw matmuls run a chunk ahead of the adj+scale finish (the LN-stats
        # chain has a full chunk of PE work to hide under)
        pend_q = []

        def finish_q(ch, pss, rr_bf):
            sl = slice(ch * 512, (ch + 1) * 512)
            broadcast_rstd(rr_bf, rxB, sl, ps_bcx)
            for m in range(FC // P):
                nc.tensor.matmul(pss[m], adjq_w[:, m * P:(m + 1) * P],
                                 adjx[:, sl], start=False, stop=True)
                nc.vector.tensor_mul(qT[:, m, sl], pss[m], rxB[:, sl])

        for ch in range(Q // 512):
            sl = slice(ch * 512, (ch + 1) * 512)
            rr_bf = ln_stats_chunk(xT, sl, adjx, ps_sx)
            pss = []
            for m in range(FC // P):
                ps = ps_qk.tile([P, 512], F32, name="ps_qk_t", tag="ps_qk_t")
                for k in range(KX):
                    nc.tensor.matmul(ps, wq_sb[:, k, m * P:(m + 1) * P],
                                     xT[:, k, sl], start=(k == 0), stop=False)
                pss.append(ps)
            pend_q.append((ch, pss, rr_bf))
            if len(pend_q) > 1:
                finish_q(*pend_q.pop(0))
        while pend_q:
            finish_q(*pend_q.pop(0))

    pool_bcx.close()
    pool_x.close()

    # =================== z statistics + k/v projections (interleaved) ======
    # softmax-denominator ones column
    nc.sync.dma_start(
        v_sb[:, :, :, HD:HD + 1],
        d["ones_bf"][:, 0:NST * NH].rearrange("p (a b c) -> p a b c",
                                              a=NST, c=1))

    pool_bcz = _Pool(tc.tile_pool(name="bcz", bufs=1))
    rzB = pool_bcz.tile([P, S], F32, tag="rzB")
    with tc.tile_pool(name="wkv", bufs=1) as pool_wkv, \
         tc.tile_pool(name="ps_sz", bufs=1, space="PSUM") as ps_sz, \
         tc.tile_pool(name="ps_bcz", bufs=1, space="PSUM") as ps_bcz, \
         tc.tile_pool(name="ps_k", bufs=2, space="PSUM") as ps_k, \
         tc.tile_pool(name="ps_v", bufs=2, space="PSUM") as ps_v:
        wk_sb = pool_wkv.tile([P, KX, FC], BF16, tag="wk_sb")
        wv_sb = pool_wkv.tile([P, KX, FC], BF16, tag="wv_sb")
        for k in range(KX):
            nc.sync.dma_start(wk_sb[:, k, :], d["wkT"][:, k, :])
            nc.sync.dma_start(wv_sb[:, k, :], d["wvT"][:, k, :])
        adjk_w = pool_wkv.tile([2, FC], F32R, tag="adjk_w")
        nc.sync.dma_start(adjk_w, d["adjk"])
        adjv_w = pool_wkv.tile([2, FC], F32R, tag="adjv_w")
        nc.sync.dma_start(adjv_w, d["adjv"])

        def v_raw(t):
            ps = ps_v.tile([P, FC], F32, name="ps_v_t", tag="ps_v_t")
            for k in range(KX):
                nc.tensor.matmul(ps, zT[:, k, t * P:(t + 1) * P],
                                 wv_sb[:, k, :], start=(k == 0), stop=False)
            return ps

        def v_fin(t, ps):
            nc.tensor.matmul(ps, adjz[:, t * P:(t + 1) * P],
                             adjv_w, start=False, stop=True)
            nc.vector.tensor_scalar_mul(
                v_sb[:, t, :, 0:HD],
                ps.rearrange("p (h e) -> p h e", h=NH),
                rz_col[:, t:t + 1])

        for ch in range(S // 512):
            sl = slice(ch * 512, (ch + 1) * 512)
            rr_bf = ln_stats_chunk(zT, sl, adjz, ps_sz, scr=rs_scr)
            # restripe this chunk's 1/std row into token-striped columns
            nc.sync.dma_start(
                rz_col[:, ch * 4:(ch + 1) * 4],
                rs_scr[0:1, sl].rearrange("a (i p) -> (a p) i", p=P))
            # raw k/v matmuls cover the stats chain; adj+scale finish after
            pss = []
            for m in range(FC // P):
                ps = ps_k.tile([P, 512], F32, name="ps_k_t", tag="ps_k_t")
                for k in range(KX):
                    nc.tensor.matmul(ps, wk_sb[:, k, m * P:(m + 1) * P],
                                     zT[:, k, sl], start=(k == 0), stop=False)
                pss.append(ps)
            vps = [v_raw(ch * 4 + ti) for ti in range(2)]
            broadcast_rstd(rr_bf, rzB, sl, ps_bcz)
            for m in range(FC // P):
                nc.tensor.matmul(pss[m], adjk_w[:, m * P:(m + 1) * P],
                                 adjz[:, sl], start=False, stop=True)
                nc.vector.tensor_mul(kT[:, m, sl], pss[m], rzB[:, sl])
            for ti in range(2):
                v_fin(ch * 4 + ti, vps[ti])
            vps = [v_raw(ch * 4 + ti) for ti in range(2, 4)]
            for ti in range(2):
                v_fin(ch * 4 + 2 + ti, vps[ti])

    pool_bcz.close()
    pool_z.close()
    pool_adj.close()
    pool_sq.close()
    pool_rows.close()

    # =================== attention ===================
    att64 = [pool_att.tile([HD, Q], BF16, name=f"att64_{h}", tag=f"att64_{h}")
             for h in range(NH)]

    with tc.tile_pool(name="mask", bufs=20) as pool_mask, \
         tc.tile_pool(name="probs", bufs=6) as pool_probs, \
         tc.tile_pool(name="rrow", bufs=2) as pool_rrow, \
         tc.tile_pool(name="ps_lg", bufs=2, space="PSUM") as ps_lg, \
         tc.tile_pool(name="ps_att", bufs=1, space="PSUM") as ps_att, \
         tc.tile_pool(name="ps_nbc", bufs=1, space="PSUM") as ps_nbc:

        steps = [(h, st) for h in range(NH) for st in range(NST)]
        att_ps = [None]         # current head's PSUM accumulator (list for closure)
        prs = {}                # g -> probs tile

        def emit_front(g):
            h, st = steps[g]
            mk = pool_mask.tile([P, Q], BF16, name="mk", tag="mk")
            nc.sync.dma_start(mk, d["maskT"][h, st * P:(st + 1) * P, :])
            lg = ps_lg.tile([P, Q], F32, name="lg", tag="lg")
            for ch in range(Q // 512):
                sl = slice(ch * 512, (ch + 1) * 512)
                ht, ho = h // 2, HD * (h % 2)
                nc.tensor.matmul(lg[:, sl],
                                 kT[ho:ho + HD, ht, st * P:(st + 1) * P],
                                 qT[ho:ho + HD, ht, sl])
            pr = pool_probs.tile([P, Q], BF16, name="pr", tag="pr")
            nc.vector.tensor_add(pr, lg, mk)
            nc.scalar.activation(pr, pr, AF.Exp)
            prs[g] = pr

        def emit_back(g):
            h, st = steps[g]
            if st == 0:
                att_ps[0] = ps_att.tile([HD + 1, Q], F32, name="att_ps",
                                        tag="att_ps")
            pr = prs.pop(g)
            for ch in range(Q // 512):
                sl = slice(ch * 512, (ch + 1) * 512)
                nc.tensor.matmul(att_ps[0][:, sl], v_sb[:, st, h, :],
                                 pr[:, sl], start=(st == 0),
                                 stop=(st == NST - 1))
            if st == NST - 1:
                # normalize: att[0:64] * broadcast(1 / att[64]).  The
                # denominator row must bounce through SBUF: the approx
                # reciprocal (custom DVE op) misreads PSUM operands on HW.
                ap = att_ps[0]
                den = pool_rrow.tile([1, Q], F32, name="den", tag="den")
                nc.vector.tensor_copy(den, ap[HD:HD + 1, :])
                rr = pool_rrow.tile([1, Q], F32, name="rr2", tag="rr2")
                nc.vector.reciprocal_approx_fast(rr, den)
                rr_bf = pool_rrow.tile([1, Q], BF16, name="rrb2", tag="rrb2")
                nc.scalar.copy(rr_bf, rr)
                nbc = ps_nbc.tile([HD, Q], F32, name="nbc", tag="nbc")
                nbs = pool_rrow.tile([HD, Q], F32, name="nbs", tag="nbs")
                for ch in range(Q // 512):
                    sl = slice(ch * 512, (ch + 1) * 512)
                    nc.tensor.matmul(nbc[:, sl], ones_bf[0:1, 0:HD],
                                     rr_bf[:, sl])
                nc.scalar.copy(nbs, nbc)
                nc.vector.tensor_mul(att64[h], ap[0:HD, :], nbs)

        DEPTH = 2
        for g in range(len(steps) + DEPTH):
            if g < len(steps):
                emit_front(g)
            if g - DEPTH >= 0:
                emit_back(g - DEPTH)

    if "dbg_q" in d:
        nc.sync.dma_start(d["dbg_q"], qT)
        nc.sync.dma_start(d["dbg_k"], kT)
        nc.sync.dma_start(d["dbg_v"], v_sb)
        for h in range(NH):
            nc.sync.dma_start(d["dbg_att"][h], att64[h])

    pool_qkv.close()

    # =================== out-projection + pipelined ReduceScatter ==========
    MC = KX // RSCH            # m-tiles per RS chunk (2)
    rs_in = [dram.tile([4, MC * P, QS], BF16, name=f"rs_in{c}", tag=f"rs_in{c}")
             for c in range(RSCH)]
    rs_out = [dram.tile([MC * P, QS], BF16, name=f"rs_out{c}", tag=f"rs_out{c}")
              for c in range(RSCH)]

    with tc.tile_pool(name="ow", bufs=1) as pool_ow, \
         tc.tile_pool(name="osb", bufs=3) as pool_osb, \
         tc.tile_pool(name="ps_o", bufs=3, space="PSUM") as ps_o:
        ow_sb = pool_ow.tile([HD, NH, D], BF16, tag="ow_sb")
        for h in range(NH):
            nc.sync.dma_start(ow_sb[:, h, :], d["owh"][:, h, :])
        for c in range(RSCH):
            for mi in range(MC):
                m = c * MC + mi
                for ch in range(Q // 512):
                    sl = slice(ch * 512, (ch + 1) * 512)
                    ps = ps_o.tile([P, 512], F32, name="ps_o_t", tag="ps_o_t")
                    for h in range(NH):
                        nc.tensor.matmul(ps, ow_sb[:, h, m * P:(m + 1) * P],
                                         att64[h][:, sl],
                                         start=(h == 0), stop=(h == NH - 1))
                    ot = pool_osb.tile([P, 512], BF16, name="ot", tag="ot")
                    nc.vector.tensor_copy(ot, ps)
                    for r2 in range(2):
                        nc.sync.dma_start(
                            rs_in[c][2 * ch + r2, mi * P:(mi + 1) * P, :],
                            ot[:, r2 * QS:(r2 + 1) * QS])
            nc.gpsimd.collective_compute(
                "ReduceScatter",
                ALU.add,
                replica_groups=REPLICA_GROUPS,
                ins=[rs_in[c].opt()],
                outs=[rs_out[c].opt()],
            )

    pool_att.close()

    # =================== residual + FFN (sequence-parallel) ===================
    with tc.tile_pool(name="ffn", bufs=1) as pool_f, \
         tc.tile_pool(name="w1s", bufs=16) as pool_w1, \
         tc.tile_pool(name="w2s", bufs=8) as pool_w2, \
         tc.tile_pool(name="gact", bufs=3) as pool_g, \
         tc.tile_pool(name="rsld", bufs=4) as pool_rsld, \
         tc.tile_pool(name="yout", bufs=3) as pool_yo:

        y1T = pool_f.tile([P, KX, QS], F32, tag="y1T")
        y1b = pool_f.tile([P, KX, QS], BF16, tag="y1b")
        adjy = pool_f.tile([2, QS], F32R, tag="adjy")     # [my ; stdy]
        ry_row = pool_f.tile([1, QS], F32, tag="ry_row")
        ry_bf = pool_f.tile([1, QS], BF16, tag="ry_bf")
        ryB = pool_f.tile([P, QS], F32, tag="ryB")
        adjf_w = pool_f.tile([2, FF], F32R, tag="adjf_w")
        nc.sync.dma_start(adjf_w, d["adjf"])

        # y1 = RS(out-proj partials) + x_slice + out_b   (feature-major)
        # gpsimd DMAs: keep the HWDGE queues free while collectives drain
        for c in range(RSCH):
            for ki in range(MC):
                k = c * MC + ki
                rst = pool_rsld.tile([P, QS], BF16, name="rst", tag="rst")
                nc.gpsimd.dma_start(rst, rs_out[c][ki * P:(ki + 1) * P, :])
                xqt = pool_rsld.tile([P, QS], F32, name="xqt", tag="xqt")
                nc.gpsimd.dma_start(xqt, d["xq"][k * P:(k + 1) * P, :])
                nc.vector.scalar_tensor_tensor(
                    out=y1T[:, k, :], in0=rst, scalar=outb_col[:, k:k + 1],
                    in1=xqt, op0=ALU.add, op1=ALU.add)
                nc.scalar.copy(y1b[:, k, :], y1T[:, k, :])
        if "dbg_y1" in d:
            nc.sync.dma_start(d["dbg_y1"], y1T)

        # y1 LN stats
        with tc.tile_pool(name="ps_yst", bufs=1, space="PSUM") as ps_yst:
            e2_row = pool_f.tile([1, QS], F32, tag="e2y_row")
            m2_row = pool_f.tile([1, QS], F32, tag="m2y_row")
            inv_row = pool_f.tile([1, QS], F32R, tag="invy_row")
            ps_sum = ps_yst.tile([1, QS], F32, name="ps_sum2", tag="ps_sum2")
            ps_ssq = ps_yst.tile([1, QS], F32, name="ps_ssq2", tag="ps_ssq2")
            for k in range(KX):
                nc.tensor.matmul(ps_sum, ones_col, y1b[:, k, :],
                                 start=(k == 0), stop=(k == KX - 1))
                sq = pool_g.tile([P, QS], BF16, name="ysq", tag="gt")
                nc.scalar.square(sq, y1b[:, k, :])
                nc.tensor.matmul(ps_ssq, ones_col, sq,
                                 start=(k == 0), stop=(k == KX - 1))
            nc.vector.tensor_scalar_mul(adjy[0:1, :], ps_sum, 1.0 / D)
            nc.vector.tensor_scalar_mul(e2_row, ps_ssq, 1.0 / D)
            nc.vector.tensor_mul(m2_row, adjy[0:1, :], adjy[0:1, :])
            nc.vector.tensor_sub(e2_row, e2_row, m2_row)
            nc.scalar.activation(inv_row, e2_row, AF.Sqrt, bias=eps_t[0:1])
            nc.vector.reciprocal_approx_fast(ry_row, inv_row.bitcast(F32))
            nc.sync.dma_start(adjy[1:2, :], inv_row)
            nc.scalar.copy(ry_bf, ry_row)
            bc = ps_yst.tile([P, QS], F32, name="bc3", tag="bc3")
            nc.tensor.matmul(bc, ones_row, ry_bf)
            nc.scalar.copy(ryB, bc)

        # ff1 (gelu) into a persistent activation tile; ff2 runs two j-steps
        # behind over half of D (PSUM accumulation groups must not share a
        # bank: each m-tile needs its own bank, so 4 at a time), then the
        # second half-D pass streams g again.
        g_sb = pool_f.tile([P, FF // P, QS], BF16, tag="g_sb")
        DH = D // 2
        ps_ff = tc.tile_pool(name="ps_ff", bufs=2, space="PSUM")
        ps_f = ps_ff.__enter__()
        ps_y2cm = tc.tile_pool(name="ps_y2", bufs=1, space="PSUM")
        ps_y2 = ps_y2cm.__enter__()

        def alloc_y2():
            return [ps_y2.tile([P, QS], F32, name=f"y2a_{i}", tag=f"y2a_{i}",
                               bufs=1) for i in range(4)]

        def emit_ff1(j):
            w1b = pool_w1.tile([P, KX, P], BF16, name="w1b", tag="w1b")
            nc.sync.dma_start(w1b, d["w1p"][j])
            ps = ps_f.tile([P, QS], F32, name="ps_f_t", tag="ps_f_t")
            for k in range(KX):
                nc.tensor.matmul(ps, w1b[:, k, :], y1b[:, k, :],
                                 start=(k == 0), stop=False)
            nc.tensor.matmul(ps, adjf_w[:, j * P:(j + 1) * P], adjy,
                             start=False, stop=True)
            nc.vector.tensor_mul(g_sb[:, j, :], ps, ryB)  # ff1 = ry*(raw+adj)
            gelu_af = (AF.Sigmoid if os.environ.get("SIM_DEBUG_SIGMOID")
                       else AF.Gelu)
            nc.scalar.activation(g_sb[:, j, :], g_sb[:, j, :], gelu_af)

        def emit_ff2(y2a, half, j):
            w2b = pool_w2.tile([P, DH], BF16, name="w2b", tag="w2b")
            nc.sync.dma_start(
                w2b, d["w2T"][j * P:(j + 1) * P, half * DH:(half + 1) * DH])
            for mi in range(4):
                nc.tensor.matmul(y2a[mi], w2b[:, mi * P:(mi + 1) * P],
                                 g_sb[:, j, :],
                                 start=(j == 0), stop=(j == FF // P - 1))

        def emit_y2_out(y2a, half):
            for mi in range(4):
                m = half * 4 + mi
                yt = pool_yo.tile([P, QS], F32, name="yt", tag="yt")
                nc.vector.scalar_tensor_tensor(
                    out=yt, in0=y2a[mi], scalar=b2_col[:, m:m + 1],
                    in1=y1T[:, m, :], op0=ALU.add, op1=ALU.add)
                nc.sync.dma_start(d["out"][m * P:(m + 1) * P, :], yt)

        y2h0 = alloc_y2()
        for j in range(FF // P + 2):
            if j < FF // P:
                emit_ff1(j)
            if j - 2 >= 0:
                emit_ff2(y2h0, 0, j - 2)
        emit_y2_out(y2h0, 0)
        y2h1 = alloc_y2()
        for j in range(FF // P):
            emit_ff2(y2h1, 1, j)
        emit_y2_out(y2h1, 1)
        ps_y2cm.__exit__(None, None, None)
        ps_ff.__exit__(None, None, None)

    const.close()
    dram.close()


def host_prep(inputs):
    """Fold layernorm gains/biases into weights; build the 8 per-core shards."""
    f32 = np.float32
    x = np.asarray(inputs["x"], f32)
    z = np.asarray(inputs["z"], f32)
    mask = np.asarray(inputs["attn_mask"], f32)
    gq = np.asarray(inputs["gq"], np.float64)
    bq = np.asarray(inputs["bq"], np.float64)
    gkv = np.asarray(inputs["gkv"], np.float64)
    bkv = np.asarray(inputs["bkv"], np.float64)
    gff = np.asarray(inputs["gff"], np.float64)
    bff = np.asarray(inputs["bff"], np.float64)
    ipw = np.asarray(inputs["in_proj_w"], np.float64)
    ipb = np.asarray(inputs["in_proj_b"], np.float64)
    out_w = np.asarray(inputs["out_w"], f32)
    out_b = np.asarray(inputs["out_b"], f32)
    w1 = np.asarray(inputs["w1"], np.float64)
    b1 = np.asarray(inputs["b1"], np.float64)
    w2 = np.asarray(inputs["w2"], f32)
    b2 = np.asarray(inputs["b2"], f32)

    wq, wk, wv = ipw[:D], ipw[D:2 * D], ipw[2 * D:]
    pq, pk, pv = ipb[:D], ipb[D:2 * D], ipb[2 * D:]
    scale = 1.0 / np.sqrt(HD)
    wq2 = (wq * gq[None, :]) * scale
    pq2 = (wq @ bq + pq) * scale
    wk2 = wk * gkv[None, :]
    pk2 = wk @ bkv + pk
    wv2 = wv * gkv[None, :]
    pv2 = wv @ bkv + pv
    w12 = w1 * gff[None, :]
    b12 = w1 @ bff + b1

    w1T = np.ascontiguousarray(w12.T.astype(f32))                    # (D, FF)
    # packed so each hidden-block's [P, KX, P] lhsT tile set is contiguous
    w1p = np.ascontiguousarray(
        w1T.reshape(KX, P, FF // P, P).transpose(2, 1, 0, 3).astype(BF))
    adjf = np.ascontiguousarray(
        np.stack([-w12.sum(1), b12]).astype(f32))                    # (2, FF)
    w2T = np.ascontiguousarray(w2.T.astype(BF))                      # (FF, D)

    def pack_kxf(wT):  # (D, FC) -> (P, D//P, FC) in bf16
        return np.ascontiguousarray(
            wT.reshape(KX, P, FC).transpose(1, 0, 2).astype(BF))

    ident = np.eye(P, dtype=BF)

    in_maps = []
    for c in range(NCORES):
        b, hg = c // 4, c % 4
        fs = slice(FC * hg, FC * hg + FC)
        qs = slice(QS * (c % 4), QS * (c % 4) + QS)
        xTb = np.ascontiguousarray(x[b].T)                           # (D, Q)
        in_maps.append({
            "ones_bf": np.ones((P, P), BF),
            "ident_bf": ident,
            "xT": xTb.astype(BF),
            "zT": np.ascontiguousarray(z[b].T).astype(BF),
            "maskT": np.ascontiguousarray(
                mask[16 * b + NH * hg:16 * b + NH * hg + NH]
                .transpose(0, 2, 1)).astype(BF),
            "wqT": pack_kxf(np.ascontiguousarray(wq2[fs].T.astype(f32))),
            "wkT": pack_kxf(np.ascontiguousarray(wk2[fs].T.astype(f32))),
            "wvT": pack_kxf(np.ascontiguousarray(wv2[fs].T.astype(f32))),
            "adjq": np.ascontiguousarray(
                np.stack([-wq2[fs].sum(1), pq2[fs]]).astype(f32)),
            "adjk": np.ascontiguousarray(
                np.stack([-wk2[fs].sum(1), pk2[fs]]).astype(f32)),
            "adjv": np.ascontiguousarray(
                np.stack([-wv2[fs].sum(1), pv2[fs]]).astype(f32)),
            "owh": np.ascontiguousarray(
                out_w[:, fs].T.reshape(NH, HD, D).transpose(1, 0, 2)
                .astype(BF)),
            "outb": out_b,
            "xq": np.ascontiguousarray(xTb[:, qs]),
            "w1p": w1p,
            "adjf": adjf,
            "w2T": w2T,
            "b2": b2,
        })
    return in_maps


_NC_CACHE = None


def kernel(**inputs) -> np.ndarray:
    global _NC_CACHE, LAST_RESULT
    from concourse.bass_utils import run_bass_kernel_spmd

    in_maps = host_prep(inputs)
    if _NC_CACHE is None:
        _NC_CACHE = build_nc()
    res = run_bass_kernel_spmd(
        _NC_CACHE, in_maps, core_ids=list(range(NCORES)),
        trace=bool(os.environ.get("BASS_TRACE")),
    )
    LAST_RESULT = res
    out = np.empty((B, Q, D), np.float32)
    for c in range(NCORES):
        b = c // 4
        qs = slice(QS * (c % 4), QS * (c % 4) + QS)
        out[b, qs, :] = res.results[c]["out"].T
    return out
